# revision 46
# baseline (speedup 1.0000x reference)
"""Trainium2 Bass kernel for a dense transformer block (self-attn + cross-attn + MLP).

Sharding: data-parallel over batch, one batch element per NeuronCore (B=8, 8 cores),
no collectives. Activations are feature-major ([C, T]) on chip.

LayerNorm is FOLDED into the projection weights on the host:
    W' = g*W - colmean(g*W),  bias' = b + W^T ln_b
so projections consume the RAW residual x (quantized straight off the stream,
no LN-apply pass); the per-token scale A[t] = 1/(128*std[t]) is applied at PSUM
evacuation (a [128,T] broadcast tile built once per LN via a ones-matmul).

Precision plan (validated in a calibrated numpy emulator, rel err ~3.8e-3):
  q,k, cross q2/k2/v2, cproj:  1-pass fp8e4m3 DoubleRow (weights x128 on host)
  v, fc:                       3-pass DoubleRow at one PSUM scale:
                               W8*x8 + W8*dx8 + R8*x8  (~bf16 accuracy, 0.75x
                               bf16 PE cost); dx8 = fp8(x - x8) via subnormals
  aproj, mproj:                bf16 (their inputs o / u are produced bf16
                               directly, avoiding on-chip hi/lo splits)
Attention interior: q/k bf16, exp/P f32r, V-aug f32r with ones column for the
softmax denominator; causal masking via one precomputed [128,896] master mask.
Softmax exp runs on paired PSUM banks ([128,1024] per ACT op); masks, squares
and fp8 deltas run on GPSIMD to keep DVE available for PSUM-coupled work.
The residual stream lives in SBUF for the whole kernel.
"""

import sys
import numpy as np

sys.path.insert(0, "/opt/trn_rl_repo")

B, T, C = 8, 1024, 1024
H = 16
D = C // H          # 64
TI = 256
FF = 4 * C          # 4096
EPS = 1e-5
NCT = C // 128      # 8 c tiles
NTT = T // 128      # 8 t tiles
P = 128
WS = 128.0          # fp8 weight scale
WSI = 1.0 / WS

_CACHED = {}

# fp8 DR packs: [MC, 128, KK*2*Mc], elem [mc, p, (kk, ko, m)] =
# q8(WS*W)[256*kk + 128*ko + p, mc*Mc + m]; *R_p carry fp8(WS*W - deq(W8)).
WPACK = {
    "attn_p": (C, 3 * C, 256),
    "vR_p": (C, C, 256),
    "q_p": (C, C, 256),
    "k_p": (C, C, 256),
    "v2_p": (C, C, 256),
    "cproj_p": (C, C, 256),
    "fc_p": (C, FF, 256),
    "fcR_p": (C, FF, 256),
}
# bf16 stationary packs: [M//128, 128, (K//128)*128], elem [mc, p, (c, m)] =
# bf16(W)[128*c + p, 128*mc + m]
BPACK = {"aproj_b": (C, C), "mproj_b": (FF, C)}


def _build(flags):
    import concourse.tile as tile
    from concourse import bacc, mybir
    from concourse.masks import make_identity

    F32, F32R = mybir.dt.float32, mybir.dt.float32r
    BF16 = mybir.dt.bfloat16
    F8 = mybir.dt.float8e4
    AF = mybir.ActivationFunctionType
    OP = mybir.AluOpType
    DR = mybir.MatmulPerfMode.DoubleRow

    qk_bias, q2_bias, ab_bias, cp_bias, mp_bias = flags

    nc = bacc.Bacc("TRN2", target_bir_lowering=False, debug=False, num_devices=8)

    dr = {}
    dr["x"] = nc.dram_tensor("x", [T, C], F32, kind="ExternalInput")
    dr["x_img_feats"] = nc.dram_tensor("x_img_feats", [TI, C], F32, kind="ExternalInput")
    for nm, shp in [
        ("b_attn", [3 * C]), ("b_aproj", [C]),
        ("bq", [C]), ("bk", [C]), ("bv", [C]), ("bcproj", [C]),
        ("b_fc", [FF]), ("b_mproj", [C]),
    ]:
        dr[nm] = nc.dram_tensor(nm, shp, F32, kind="ExternalInput")
    for nm, (K, M, Mc) in WPACK.items():
        dr[nm] = nc.dram_tensor(nm, [M // Mc, P, (K // 256) * 2 * Mc], F8,
                                kind="ExternalInput")
    for nm, (K, M) in BPACK.items():
        dr[nm] = nc.dram_tensor(nm, [M // P, P, (K // P) * P], BF16,
                                kind="ExternalInput")
    out_d = nc.dram_tensor("out", [T, C], F32, kind="ExternalOutput")

    with tile.TileContext(nc) as tc, nc.allow_low_precision(
        reason="fp8 DoubleRow projections + bf16 attention are intentional"
    ):
        kw_cms = []

        def openp(**kw):
            cm = tc.tile_pool(**kw)
            return cm, cm.__enter__()

        def openkw(**kw):
            cm, p = openp(**kw)
            kw_cms.append(cm)
            return p

        # ---------------- kernel-wide pools (left-stack base) ----------------
        constp = openkw(name="const", bufs=1)
        fsrp = openkw(name="fsr", bufs=2)       # f32r [128,512] squares
        abp = openkw(name="ab", bufs=1)         # A_b [128,1024] + A_col
        rowp = openkw(name="rows", bufs=3)      # [1,1024] rows
        rbp = openkw(name="rb", bufs=4)         # [64,512] + [1,512] rden

        # ---------------- constants ----------------
        ident = constp.tile([P, P], F32)
        make_identity(nc, ident)
        identR = constp.tile([P, P], F32R)
        nc.vector.tensor_copy(out=identR, in_=ident)

        ones_col = constp.tile([P, 16], F32)
        nc.vector.memset(ones_col, 1.0)
        ones128R = constp.tile([P, 1], F32R)
        nc.vector.tensor_copy(out=ones128R, in_=ones_col[:, 0:1])
        o1x = constp.tile([1, P], F32)
        nc.vector.memset(o1x, 1.0)
        ones_1x128 = constp.tile([1, P], F32R)
        nc.vector.tensor_copy(out=ones_1x128, in_=o1x)
        epsS_t = constp.tile([1, 1], F32)
        nc.vector.memset(epsS_t, EPS * WS * WS)
        zeros384 = constp.tile([P, 384], F32)
        nc.vector.memset(zeros384, 0.0)

        master = constp.tile([P, 896], F32)
        nc.gpsimd.memset(master, 1.0)
        nc.gpsimd.affine_select(
            out=master, in_=master, compare_op=OP.is_ge, fill=0.0,
            base=-384, pattern=[[1, 896]], channel_multiplier=-1)

        # ================= P0: load & transpose x (issued first) =============
        res_cm, residp = openp(name="resid", bufs=NCT, side="right")
        resid = [residp.tile([P, T], F32R, tag="res", name="res") for _ in range(NCT)]

        tok_cm, tokp = openp(name="tok0", bufs=4)
        tp_cm, tpp = openp(name="psT0", bufs=2, space="PSUM")
        toks = []
        for tt in range(NTT):
            tok = tokp.tile([P, C], F32, tag="tok", name="tok")
            nc.sync.dma_start(out=tok, in_=dr["x"].ap()[tt * P:(tt + 1) * P, :])
            toks.append(tok)
        for tt in range(NTT):
            tok = toks[tt]
            for c in range(NCT):
                tps = tpp.tile([P, P], F32, tag="tp", name="tp")
                nc.tensor.transpose(tps, tok[:, c * P:(c + 1) * P], ident)
                if c % 2:
                    nc.vector.tensor_copy(out=resid[c][:, tt * P:(tt + 1) * P], in_=tps)
                else:
                    nc.scalar.copy(out=resid[c][:, tt * P:(tt + 1) * P], in_=tps)
        tp_cm.__exit__(None, None, None)
        tok_cm.__exit__(None, None, None)

        # ---------------- small input rows (issued after x) ----------------
        def load_cols(name, nf):
            t = constp.tile([P, nf], F32, name=name + "_c")
            nc.sync.dma_start(out=t, in_=dr[name].ap().rearrange("(f p) -> p f", p=P))
            return t

        bqk = constp.tile([P, 16], F32)
        nc.sync.dma_start(out=bqk, in_=dr["b_attn"].ap()[0:2 * C].rearrange("(f p) -> p f", p=P))
        bq_c = load_cols("bq", NCT)
        bk_c = load_cols("bk", NCT)
        bap_c = load_cols("b_aproj", NCT)
        bcp_c = load_cols("bcproj", NCT)
        bmp_c = load_cols("b_mproj", NCT)
        bfc_c = load_cols("b_fc", FF // 128)

        # ---------------- helpers ----------------
        def load_wp(name, mc, wpool):
            K, M, Mc = WPACK[name]
            KK = K // 256
            t = wpool.tile([P, KK, 2, Mc], F8, tag="wp", name="wp")
            nc.sync.dma_start(
                out=t,
                in_=dr[name].ap()[mc].rearrange("p (kk ko m) -> p kk ko m", kk=KK, ko=2))
            return t

        def load_wb(name, mc, wpool):
            K, M = BPACK[name]
            nk = K // P
            t = wpool.tile([P, nk, P], BF16, tag="wb", name="wb")
            src = dr[name].ap()[mc].rearrange("p (c m) -> p c m", m=P)
            nc.sync.dma_start(out=t[:, 0:nk // 2, :], in_=src[:, 0:nk // 2, :])
            nc.sync.dma_start(out=t[:, nk // 2:nk, :], in_=src[:, nk // 2:nk, :])
            return t

        def bcast_row(row_f32, dest_pool, psp, tag):
            rowr = rowp.tile([1, C], F32R, tag="row", name="rowr")
            nc.vector.tensor_copy(out=rowr, in_=row_f32)
            dest = dest_pool.tile([P, C], F32, tag=tag, name=tag)
            for cc in range(2):
                bps = psp.tile([P, 512], F32, tag="bc", name="bc")
                nc.tensor.matmul(bps, ones_1x128, rowr[:, 512 * cc:512 * (cc + 1)],
                                 start=True, stop=True)
                nc.scalar.copy(out=dest[:, 512 * cc:512 * (cc + 1)], in_=bps)
            return dest

        def ln_stats(xtiles, psp, with_col=False):
            """A_b [128,T] broadcast of A[t] = 1/(128*std[t]); opt A_col [128,NTT]."""
            sum_ps, sq_ps = [], []
            for tch in range(2):
                sp = psp.tile([1, 512], F32, tag="lnsum", name="lnsum")
                qp = psp.tile([1, 512], F32, tag="lnsq", name="lnsq")
                for c in range(NCT):
                    xs = xtiles[c][:, 512 * tch:512 * (tch + 1)]
                    nc.tensor.matmul(sp, ones128R, xs, start=(c == 0), stop=(c == NCT - 1))
                    sq = fsrp.tile([P, 512], F32R, tag="sq", name="sq")
                    nc.scalar.activation(out=sq, in_=xs, func=AF.Square, scale=1.0)
                    nc.tensor.matmul(qp, ones128R, sq, start=(c == 0), stop=(c == NCT - 1))
                sum_ps.append(sp)
                sq_ps.append(qp)
            mu = rowp.tile([1, T], F32, tag="row", name="mu")
            for tch in range(2):
                nc.vector.tensor_scalar_mul(out=mu[:, 512 * tch:512 * (tch + 1)],
                                            in0=sum_ps[tch], scalar1=1.0 / C)
            musq = rowp.tile([1, T], F32, tag="row", name="musq")
            nc.vector.tensor_tensor(out=musq, in0=mu, in1=mu, op=OP.mult)
            msq = rowp.tile([1, T], F32, tag="row", name="msq")
            for tch in range(2):
                sl = slice(512 * tch, 512 * (tch + 1))
                nc.vector.scalar_tensor_tensor(
                    out=msq[:, sl], in0=sq_ps[tch], scalar=1.0 / C,
                    in1=musq[:, sl], op0=OP.mult, op1=OP.subtract)
            nc.scalar.activation(out=musq, in_=msq, func=AF.Sqrt, bias=epsS_t,
                                 scale=WS * WS)
            arow = rowp.tile([1, T], F32R, tag="row", name="arow")
            nc.vector.reciprocal(out=arow, in_=musq)
            A_b = abp.tile([P, T], F32, tag="A_b", name="A_b")
            for tch in range(2):
                sl = slice(512 * tch, 512 * (tch + 1))
                bps = psp.tile([P, 512], F32, tag="bc", name="bc")
                nc.tensor.matmul(bps, ones_1x128, arow[:, sl], start=True, stop=True)
                nc.scalar.copy(out=A_b[:, sl], in_=bps)
            if not with_col:
                return A_b, None
            A_col = abp.tile([P, NTT], F32, tag="A_col", name="A_col")
            for tt in range(NTT):
                cps = psp.tile([P, P], F32, tag="bc", name="bc")
                nc.tensor.transpose(cps, A_b[:, tt * P:(tt + 1) * P], ident)
                nc.vector.tensor_copy(out=A_col[:, tt:tt + 1], in_=cps[:, 0:1])
            return A_b, A_col

        def quant_x(xtiles, x8, xd8, pool_only=False):
            """fp8 copy of the residual stream (+ optional fp8 delta)."""
            for tch in range(2):
                sl = slice(512 * tch, 512 * (tch + 1))
                for c in range(NCT):
                    eng = nc.gpsimd if (pool_only or c % 2 == 0) else nc.vector
                    eng.tensor_copy(out=x8[:, c, sl], in_=xtiles[c][:, sl])
            if xd8 is None:
                return
            for tch in range(2):
                sl = slice(512 * tch, 512 * (tch + 1))
                for c in range(NCT):
                    eng = nc.gpsimd if (pool_only or c % 2 == 0) else nc.vector
                    eng.scalar_tensor_tensor(
                        out=xd8[:, c, sl], in0=x8[:, c, sl], scalar=-1.0,
                        in1=xtiles[c][:, sl], op0=OP.mult, op1=OP.add)

        def attn_chunk(kq_of, vaug_tiles, n_s, h, tch, psp, ppool, causal,
                       o_all, rb_split=True, s_bufs=3, o_bufs=2):
            (kt, ko), (qt, qo) = kq_of(h)
            tsl = slice(512 * tch, 512 * (tch + 1))
            ptiles = []
            pair_ps = []
            for pr in range(n_s // 2):
                sps = psp.tile([P, 1024], F32, tag="s", name="s", bufs=s_bufs)
                for hf in range(2):
                    st = 2 * pr + hf
                    nc.tensor.matmul(sps[:, 512 * hf:512 * hf + 512],
                                     kt[ko:ko + D, st * P:(st + 1) * P],
                                     qt[qo:qo + D, tsl], start=True, stop=True,
                                     tile_position=(ko, 0))
                pair_ps.append(sps)
            for pr in range(n_s // 2):
                sps = pair_ps[pr]
                pt = ppool.tile([P, 1024], F32R, tag="p", name="p")
                j0 = 2 * pr - 4 * tch
                j1 = j0 + 1
                d0 = causal and j0 >= 0
                d1 = causal and j1 >= 0
                z0 = P * j0 if d0 else 0
                z1 = P * j1 if d1 else 0
                nc.scalar.activation(out=pt[:, z0:1024], in_=sps[:, z0:1024],
                                     func=AF.Exp, scale=0.125)
                if d0 and z0:
                    nc.gpsimd.tensor_copy(out=pt[:, 0:z0], in_=zeros384[:, 0:z0])
                if d1 and z1:
                    nc.gpsimd.tensor_copy(out=pt[:, 512:512 + z1], in_=zeros384[:, 0:z1])
                if d0:
                    nc.gpsimd.tensor_tensor(out=pt[:, z0:z0 + P], in0=pt[:, z0:z0 + P],
                                            in1=master[:, 384:512], op=OP.mult)
                if d1:
                    nc.gpsimd.tensor_tensor(out=pt[:, 512 + z1:512 + z1 + P],
                                            in0=pt[:, 512 + z1:512 + z1 + P],
                                            in1=master[:, 384:512], op=OP.mult)
                ptiles.append(pt)
            ops = psp.tile([P, 512], F32, tag="o", name="o", bufs=o_bufs)
            for st in range(n_s):
                pt = ptiles[st // 2][:, 512 * (st % 2):512 * (st % 2) + 512]
                nc.tensor.matmul(ops[0:65, :], vaug_tiles[st][:, 65 * h:65 * h + 65],
                                 pt, start=(st == 0), stop=(st == n_s - 1))
            rden = rbp.tile([1, 512], F32R, tag="rden", name="rden")
            nc.vector.reciprocal(out=rden, in_=ops[64:65, :])
            # broadcast 1/den into the unused partitions 64..127 of the same
            # PSUM tile (WAR on the den row orders this after the reciprocal)
            nc.tensor.matmul(ops[64:128, :], ones_1x128[:, 0:64], rden,
                             start=True, stop=True, tile_position=(0, 64))
            rb = rbp.tile([64, 512], F32, tag="rb", name="rb")
            if rb_split and h % 2:
                nc.vector.tensor_copy(out=rb, in_=ops[64:128, :])
            else:
                nc.scalar.copy(out=rb, in_=ops[64:128, :])
            po = (h % 2) * D
            nc.vector.tensor_tensor(out=o_all[po:po + D, h // 2, tsl],
                                    in0=ops[0:64, :], in1=rb, op=OP.mult)

        def dr_group(psum, pairs):
            n = len(pairs)
            for i, (lh, rh) in enumerate(pairs):
                nc.tensor.matmul(psum, lh, rh, start=(i == 0), stop=(i == n - 1),
                                 perf_mode=DR)

        def ws_passes(wt, wtR, h8, hd, msl, tsl2):
            ps = [(wt[:, kk, :, msl], h8[:, 2 * kk:2 * kk + 2, tsl2]) for kk in range(4)]
            if hd is not None:
                ps += [(wt[:, kk, :, msl], hd[:, 2 * kk:2 * kk + 2, tsl2]) for kk in range(4)]
            if wtR is not None:
                ps += [(wtR[:, kk, :, msl], h8[:, 2 * kk:2 * kk + 2, tsl2]) for kk in range(4)]
            return ps

        # ================= P1: LN1 + qkv projections =================
        x8_cm, x8p = openp(name="x8", bufs=1)
        x8 = x8p.tile([P, NCT, T], F8, tag="x8", name="x8")

        xd8_cm, xd8p = openp(name="xd8", bufs=1)
        xd8 = xd8p.tile([P, NCT, T], F8, tag="xd8", name="xd8")

        ln_cm, lnp = openp(name="psLN0", bufs=2, space="PSUM")
        A_b, A_col = ln_stats(resid, lnp, with_col=True)
        ln_cm.__exit__(None, None, None)
        quant_x(resid, x8, xd8)

        # ---- cross-attention K/V side (depends only on x_img_feats): hoisted
        # into the startup window so its latency chains overlap qkv compute.
        cross_cm, crossp = openp(name="cross", bufs=12, side="right")
        imgT = crossp.tile([P, NCT, TI], F8, tag="imgT", name="imgT", bufs=1)
        k2_t = [crossp.tile([P, TI], BF16, tag="k2", name="k2", bufs=NCT)
                for _ in range(NCT)]
        v2aug = [crossp.tile([P, 16 * 65], F32R, tag="va2", name="va2", bufs=2)
                 for _ in range(TI // P)]

        wk_cm, wk = openp(name="wk", bufs=3)
        tok_cm, tokp = openp(name="tok4", bufs=2)
        acckv_cm, acckv = openp(name="psKV", bufs=2, space="PSUM")
        tpi_cm, tpi = openp(name="psT4", bufs=2, space="PSUM")
        for tt in range(TI // P):
            tok = tokp.tile([P, C], F32, tag="tok", name="tok")
            nc.sync.dma_start(out=tok, in_=dr["x_img_feats"].ap()[tt * P:(tt + 1) * P, :])
            for c in range(NCT):
                tps = tpi.tile([P, P], F32, tag="tp", name="tp")
                nc.tensor.transpose(tps, tok[:, c * P:(c + 1) * P], ident)
                nc.vector.tensor_copy(out=imgT[:, c, tt * P:(tt + 1) * P], in_=tps)
        tpi_cm.__exit__(None, None, None)
        tok_cm.__exit__(None, None, None)

        for mc in range(4):
            wt = load_wp("k_p", mc, wk)
            for mh in range(2):
                f = 2 * mc + mh
                kps = acckv.tile([P, 256], F32, tag="acc256", name="acc256")
                dr_group(kps, [(wt[:, kk, :, 128 * mh:128 * mh + 128],
                                imgT[:, 2 * kk:2 * kk + 2, :]) for kk in range(4)])
                nc.scalar.activation(out=k2_t[f], in_=kps, func=AF.Identity,
                                     bias=bk_c[:, f:f + 1], scale=WSI)

        brow_v2 = rowp.tile([1, C], F32, tag="row", name="braw2")
        nc.sync.dma_start(out=brow_v2, in_=dr["bv"].ap().rearrange("(a c) -> a c", a=1))
        bvb2 = bcast_row(brow_v2, wk, acckv, "bvb2")
        for cc in range(4):
            wt = load_wp("v2_p", cc, wk)
            for st in range(TI // P):
                vps = acckv.tile([P, 256], F32, tag="acc256", name="acc256")
                dr_group(vps, [(imgT[:, 2 * kk:2 * kk + 2, st * P:(st + 1) * P],
                                wt[:, kk, :, :]) for kk in range(4)])
                dst = v2aug[st].rearrange("p (h x) -> p h x", x=65)[:, 4 * cc:4 * (cc + 1), 0:64]
                nc.vector.scalar_tensor_tensor(
                    out=dst, in0=vps.rearrange("p (h x) -> p h x", x=64),
                    scalar=WSI,
                    in1=bvb2[:, 256 * cc:256 * (cc + 1)].rearrange("p (h x) -> p h x", x=64),
                    op0=OP.mult, op1=OP.add)
        for st in range(TI // P):
            nc.vector.tensor_copy(
                out=v2aug[st].rearrange("p (h x) -> p h x", x=65)[:, :, 64:65],
                in_=ones_col.rearrange("p (h x) -> p h x", x=1))
        wk_cm.__exit__(None, None, None)

        # ---- self-attention v projection (3-pass) + q,k (1-pass)
        vap_cm, vap = openp(name="vaug", bufs=NTT, side="right")
        vaug = [vap.tile([P, 16 * 65], F32R, tag="va", name="va") for _ in range(NTT)]

        wv_cm, wv = openp(name="wv", bufs=4)
        accv_cm, accv = openp(name="psACv", bufs=2, space="PSUM")
        brow_v = rowp.tile([1, C], F32, tag="row", name="braw")
        nc.sync.dma_start(out=brow_v,
                          in_=dr["b_attn"].ap()[2 * C:3 * C].rearrange("(a c) -> a c", a=1))
        bvb1 = bcast_row(brow_v, wv, accv, "bvb")
        for cc in range(4):   # v output chunks of 256 cols (4 heads each)
            wt = load_wp("attn_p", 8 + cc, wv)
            wtR = load_wp("vR_p", cc, wv)
            for tt in range(NTT):
                vps = accv.tile([P, 256], F32, tag="acc", name="acc")
                tsl = slice(tt * P, (tt + 1) * P)
                ps = ([(x8[:, 2 * kk:2 * kk + 2, tsl], wt[:, kk, :, :]) for kk in range(4)]
                      + [(xd8[:, 2 * kk:2 * kk + 2, tsl], wt[:, kk, :, :]) for kk in range(4)]
                      + [(x8[:, 2 * kk:2 * kk + 2, tsl], wtR[:, kk, :, :]) for kk in range(4)])
                dr_group(vps, ps)
                dst = vaug[tt].rearrange("p (h x) -> p h x", x=65)[:, 4 * cc:4 * (cc + 1), 0:64]
                nc.vector.scalar_tensor_tensor(
                    out=dst, in0=vps.rearrange("p (h x) -> p h x", x=64),
                    scalar=A_col[:, tt:tt + 1],
                    in1=bvb1[:, 256 * cc:256 * (cc + 1)].rearrange("p (h x) -> p h x", x=64),
                    op0=OP.mult, op1=OP.add)
        for tt in range(NTT):
            nc.vector.tensor_copy(
                out=vaug[tt].rearrange("p (h x) -> p h x", x=65)[:, :, 64:65],
                in_=ones_col.rearrange("p (h x) -> p h x", x=1))
        accv_cm.__exit__(None, None, None)
        wv_cm.__exit__(None, None, None)
        acckv_cm.__exit__(None, None, None)
        xd8_cm.__exit__(None, None, None)

        qk_cm, qkp = openp(name="qk", bufs=16, side="right")
        w1_cm, w1 = openp(name="w1", bufs=3)
        acc_cm, accp = openp(name="psAC1", bufs=4, space="PSUM")
        qk_t = []
        for mc in range(8):
            wt = load_wp("attn_p", mc, w1)
            for mh in range(2):
                f = 2 * mc + mh
                qt = qkp.tile([P, T], BF16, tag="qk", name="qk")
                for tch in range(2):
                    sl = slice(512 * tch, 512 * (tch + 1))
                    aps = accp.tile([P, 512], F32, tag="acc", name="acc")
                    dr_group(aps, ws_passes(wt, None, x8, None,
                                            slice(128 * mh, 128 * mh + 128), sl))
                    nc.vector.tensor_tensor(out=qt[:, sl], in0=aps, in1=A_b[:, sl],
                                            op=OP.mult)
                    if qk_bias:
                        nc.vector.tensor_scalar_add(out=qt[:, sl], in0=qt[:, sl],
                                                    scalar1=bqk[:, f:f + 1])
                qk_t.append(qt)
        acc_cm.__exit__(None, None, None)
        w1_cm.__exit__(None, None, None)
        x8_cm.__exit__(None, None, None)

        # ================= P2: self attention =================
        o_cm, opool = openp(name="o1", bufs=1)
        o_all = opool.tile([P, NCT, T], BF16, tag="ot", name="ot")
        pp_cm, pp = openp(name="pp1", bufs=5)
        psS_cm, psS = openp(name="psS1", bufs=2, space="PSUM")

        def kq_self(h):
            return (qk_t[8 + h // 2], (h % 2) * D), (qk_t[h // 2], (h % 2) * D)

        for tch in range(2):
            for h in range(H):
                attn_chunk(kq_self, vaug, 4 * (tch + 1), h, tch, psS, pp,
                           causal=True, o_all=o_all)

        psS_cm.__exit__(None, None, None)
        pp_cm.__exit__(None, None, None)
        qk_cm.__exit__(None, None, None)
        vap_cm.__exit__(None, None, None)

        # ================= P3: aproj (bf16) + residual in place ======
        w2_cm, w2 = openp(name="w2", bufs=3)
        acc_cm, accp = openp(name="psAC3", bufs=3, space="PSUM")
        for co in range(NCT):
            wt = load_wb("aproj_b", co, w2)
            for tch in range(2):
                sl = slice(512 * tch, 512 * (tch + 1))
                aps = accp.tile([P, 512], F32, tag="acc", name="acc")
                for c in range(NCT):
                    nc.tensor.matmul(aps, wt[:, c, :], o_all[:, c, sl],
                                     start=(c == 0), stop=(c == NCT - 1))
                nc.vector.tensor_tensor(out=resid[co][:, sl], in0=aps,
                                        in1=resid[co][:, sl], op=OP.add)
                if ab_bias:
                    nc.vector.tensor_scalar_add(
                        out=resid[co][:, sl], in0=resid[co][:, sl],
                        scalar1=bap_c[:, co:co + 1])
        acc_cm.__exit__(None, None, None)
        w2_cm.__exit__(None, None, None)
        o_cm.__exit__(None, None, None)

        # ================= P4: cross attention projections =================
        x1_cm, x1p = openp(name="x18", bufs=1)
        x18 = x1p.tile([P, NCT, T], F8, tag="x8", name="x8")

        ln_cm, lnp = openp(name="psLN1", bufs=2, space="PSUM")
        A_b, _ = ln_stats(resid, lnp)
        ln_cm.__exit__(None, None, None)
        quant_x(resid, x18, None, pool_only=True)

        w3_cm, w3 = openp(name="w3", bufs=3)
        acc_cm, accp = openp(name="psAC4", bufs=4, space="PSUM")
        q2_cm, q2p = openp(name="q2", bufs=NCT, side="right")
        q2_t = []
        for mc in range(4):
            wt = load_wp("q_p", mc, w3)
            for mh in range(2):
                f = 2 * mc + mh
                qt = q2p.tile([P, T], BF16, tag="q2", name="q2")
                for tch in range(2):
                    sl = slice(512 * tch, 512 * (tch + 1))
                    aps = accp.tile([P, 512], F32, tag="acc", name="acc")
                    dr_group(aps, ws_passes(wt, None, x18, None,
                                            slice(128 * mh, 128 * mh + 128), sl))
                    nc.vector.tensor_tensor(out=qt[:, sl], in0=aps, in1=A_b[:, sl],
                                            op=OP.mult)
                    if q2_bias:
                        nc.vector.tensor_scalar_add(out=qt[:, sl], in0=qt[:, sl],
                                                    scalar1=bq_c[:, f:f + 1])
                q2_t.append(qt)
        acc_cm.__exit__(None, None, None)
        w3_cm.__exit__(None, None, None)
        x1_cm.__exit__(None, None, None)

        # ================= P5: cross attention =================
        o_cm, opool = openp(name="o2", bufs=1)
        o2_all = opool.tile([P, NCT, T], F8, tag="ot", name="ot")
        pp_cm, pp = openp(name="pp2", bufs=4)
        psS_cm, psS = openp(name="psS2", bufs=2, space="PSUM")

        def kq_cross(h):
            return (k2_t[h // 2], (h % 2) * D), (q2_t[h // 2], (h % 2) * D)

        for tch in range(2):
            for h in range(H):
                attn_chunk(kq_cross, v2aug, TI // P, h, tch, psS, pp,
                           causal=False, o_all=o2_all, rb_split=False,
                           s_bufs=3, o_bufs=2)

        psS_cm.__exit__(None, None, None)
        pp_cm.__exit__(None, None, None)
        q2_cm.__exit__(None, None, None)
        cross_cm.__exit__(None, None, None)

        # ================= P6: cproj + residual (x2, in place) =================
        w4_cm, w4 = openp(name="w4", bufs=3)
        acc_cm, accp = openp(name="psAC5", bufs=3, space="PSUM")
        for mc in range(4):
            wt = load_wp("cproj_p", mc, w4)
            for mh in range(2):
                co = 2 * mc + mh
                for tch in range(2):
                    sl = slice(512 * tch, 512 * (tch + 1))
                    aps = accp.tile([P, 512], F32, tag="acc", name="acc")
                    dr_group(aps, ws_passes(wt, None, o2_all, None,
                                            slice(128 * mh, 128 * mh + 128), sl))
                    nc.vector.scalar_tensor_tensor(
                        out=resid[co][:, sl], in0=aps, scalar=WSI,
                        in1=resid[co][:, sl], op0=OP.mult, op1=OP.add)
                    if cp_bias:
                        nc.vector.tensor_scalar_add(
                            out=resid[co][:, sl], in0=resid[co][:, sl],
                            scalar1=bcp_c[:, co:co + 1])
        acc_cm.__exit__(None, None, None)
        w4_cm.__exit__(None, None, None)
        o_cm.__exit__(None, None, None)

        # ================= P7: MLP =================
        x2_cm, x2p = openp(name="x28", bufs=1)
        x28 = x2p.tile([P, NCT, T], F8, tag="x8", name="x8")
        x2d8 = x2p.tile([P, NCT, T], F8, tag="xd8", name="xd8")

        ln_cm, lnp = openp(name="psLN2", bufs=2, space="PSUM")
        A_b, _ = ln_stats(resid, lnp)
        ln_cm.__exit__(None, None, None)
        quant_x(resid, x28, x2d8)

        up_cm, up = openp(name="u", bufs=16, side="right")
        utiles = [up.tile([P, 2, T], BF16, tag="u", name="u") for _ in range(16)]
        uscr_cm, uscrp = openp(name="uscr", bufs=4)
        w5_cm, w5 = openp(name="w5", bufs=4)
        accU_cm, accU = openp(name="psU", bufs=4, space="PSUM")
        for mc in range(16):
            wt = load_wp("fc_p", mc, w5)
            wtR = load_wp("fcR_p", mc, w5)
            for mh in range(2):
                ff = 2 * mc + mh
                for tch in range(2):
                    sl = slice(512 * tch, 512 * (tch + 1))
                    ups = accU.tile([P, 512], F32, tag="acc", name="acc")
                    dr_group(ups, ws_passes(wt, wtR, x28, x2d8,
                                            slice(128 * mh, 128 * mh + 128), sl))
                    uscr = uscrp.tile([P, 512], F32, tag="us", name="us")
                    nc.vector.tensor_tensor(out=uscr, in0=ups, in1=A_b[:, sl],
                                            op=OP.mult)
                    nc.scalar.activation(out=utiles[ff // 2][:, ff % 2, sl], in_=uscr,
                                         func=AF.Gelu_apprx_tanh,
                                         bias=bfc_c[:, ff:ff + 1], scale=1.0)
        accU_cm.__exit__(None, None, None)
        w5_cm.__exit__(None, None, None)
        uscr_cm.__exit__(None, None, None)
        x2_cm.__exit__(None, None, None)

        w6_cm, w6 = openp(name="w6", bufs=3)
        psM_cm, psM = openp(name="psM", bufs=3, space="PSUM")
        for co in range(NCT):
            wt = load_wb("mproj_b", co, w6)
            for tch in range(2):
                sl = slice(512 * tch, 512 * (tch + 1))
                mps = psM.tile([P, 512], F32, tag="m", name="m")
                for ff in range(FF // P):
                    nc.tensor.matmul(mps, wt[:, ff, :], utiles[ff // 2][:, ff % 2, sl],
                                     start=(ff == 0), stop=(ff == FF // P - 1))
                nc.vector.tensor_tensor(out=resid[co][:, sl], in0=mps,
                                        in1=resid[co][:, sl], op=OP.add)
                if mp_bias:
                    nc.vector.tensor_scalar_add(
                        out=resid[co][:, sl], in0=resid[co][:, sl],
                        scalar1=bmp_c[:, co:co + 1])
        psM_cm.__exit__(None, None, None)
        w6_cm.__exit__(None, None, None)
        up_cm.__exit__(None, None, None)

        # ================= P8: transpose back & store =================
        tok_cm, tokp = openp(name="tok7", bufs=2)
        tp_cm, tpp = openp(name="psT7", bufs=4, space="PSUM")
        for tt in range(NTT):
            otok = tokp.tile([P, C], F32, tag="tok", name="tok")
            for c in range(NCT):
                tps = tpp.tile([P, P], F32R, tag="tpr", name="tpr")
                nc.tensor.transpose(tps, resid[c][:, tt * P:(tt + 1) * P], identR)
                if c % 2:
                    nc.vector.tensor_copy(out=otok[:, c * P:(c + 1) * P], in_=tps)
                else:
                    nc.scalar.copy(out=otok[:, c * P:(c + 1) * P], in_=tps)
            nc.sync.dma_start(out=out_d.ap()[tt * P:(tt + 1) * P, :], in_=otok)
        tp_cm.__exit__(None, None, None)
        tok_cm.__exit__(None, None, None)
        res_cm.__exit__(None, None, None)

        for cm in reversed(kw_cms):
            cm.__exit__(None, None, None)

    nc.compile()
    return nc


def _pack_core(Wq, Mc):
    K, M = Wq.shape
    KK, MC = K // 256, M // Mc
    A = Wq.reshape(KK, 2, P, MC, Mc).transpose(3, 2, 0, 1, 4)
    return np.ascontiguousarray(A.reshape(MC, P, KK * 2 * Mc))


def _pack_w(W, Mc=256):
    import ml_dtypes
    return _pack_core((np.asarray(W, np.float32) * WS).astype(ml_dtypes.float8_e4m3), Mc)


def _pack_wr(W, Mc=256):
    import ml_dtypes
    Ws = np.asarray(W, np.float32) * WS
    W8 = Ws.astype(ml_dtypes.float8_e4m3)
    R = Ws - W8.astype(np.float32)
    return _pack_core(R.astype(ml_dtypes.float8_e4m3), Mc)


def _pack_b(W):
    """[K, M] f32 -> [M//128, 128, (K//128)*128] bf16 stationary slabs."""
    import ml_dtypes
    K, M = W.shape
    A = np.asarray(W, np.float32).reshape(K // P, P, M // P, P).transpose(2, 1, 0, 3)
    return np.ascontiguousarray(A.reshape(M // P, P, K).astype(ml_dtypes.bfloat16))


def _fold(W, g):
    Wg = np.asarray(W, np.float32) * np.asarray(g, np.float32)[:, None]
    return Wg - Wg.mean(0, keepdims=True)


def kernel(**inputs):
    from concourse.bass_utils import run_bass_kernel_spmd

    np_inputs = {k: np.asarray(v, dtype=np.float32) for k, v in inputs.items()}
    g1, b1 = np_inputs["ln1_g"], np_inputs["ln1_b"]
    g2, b2 = np_inputs["ln2_g"], np_inputs["ln2_b"]
    Wa = np_inputs["W_attn"]

    # fold LN gains/means into weights; fold LN biases into effective biases
    Wa_f = _fold(Wa, g1)
    Wq_f = _fold(np_inputs["Wq"], g1)
    Wfc_f = _fold(np_inputs["W_fc"], g2)
    b_attn_eff = np_inputs["b_attn"] + Wa.T @ b1
    bq_eff = np_inputs["bq"] + np_inputs["Wq"].T @ b1
    bfc_eff = np_inputs["b_fc"] + np_inputs["W_fc"].T @ b2

    flags = (bool(np.any(b_attn_eff[0:2 * C])), bool(np.any(bq_eff)),
             bool(np.any(np_inputs["b_aproj"])), bool(np.any(np_inputs["bcproj"])),
             bool(np.any(np_inputs["b_mproj"])))
    key = ("nc", flags)
    if key not in _CACHED:
        _CACHED[key] = _build(flags)
    nc = _CACHED[key]

    packed = {
        "attn_p": _pack_w(Wa_f),
        "vR_p": _pack_wr(Wa_f[:, 2 * C:3 * C]),
        "q_p": _pack_w(Wq_f),
        "k_p": _pack_w(np_inputs["Wk"]),
        "v2_p": _pack_w(np_inputs["Wv"]),
        "cproj_p": _pack_w(np_inputs["Wcproj"]),
        "fc_p": _pack_w(Wfc_f),
        "fcR_p": _pack_wr(Wfc_f),
        "aproj_b": _pack_b(np_inputs["W_aproj"]),
        "mproj_b": _pack_b(np_inputs["W_mproj"]),
    }
    small = {
        "b_attn": b_attn_eff, "bq": bq_eff, "b_fc": bfc_eff,
        "b_aproj": np_inputs["b_aproj"], "bk": np_inputs["bk"],
        "bv": np_inputs["bv"], "bcproj": np_inputs["bcproj"],
        "b_mproj": np_inputs["b_mproj"],
    }
    in_maps = []
    for b in range(B):
        m = dict(small)
        m.update(packed)
        m["x"] = np.ascontiguousarray(np_inputs["x"][b])
        m["x_img_feats"] = np.ascontiguousarray(np_inputs["x_img_feats"][b])
        in_maps.append(m)
    res = run_bass_kernel_spmd(nc, in_maps, core_ids=list(range(B)))
    out = np.stack([res.results[b]["out"] for b in range(B)], axis=0)
    return out.astype(np.float32)


# revision 47
# speedup vs baseline: 1.0193x; 1.0193x over previous
"""Trainium2 Bass kernel for a dense transformer block (self-attn + cross-attn + MLP).

Sharding: data-parallel over batch, one batch element per NeuronCore (B=8, 8 cores),
no collectives. Activations are feature-major ([C, T]) on chip.

LayerNorm is FOLDED into the projection weights on the host:
    W' = g*W - colmean(g*W),  bias' = b + W^T ln_b
so projections consume the RAW residual x (quantized straight off the stream,
no LN-apply pass); the per-token scale A[t] = 1/(128*std[t]) is applied at PSUM
evacuation (a [128,T] broadcast tile built once per LN via a ones-matmul).

Precision plan (validated in a calibrated numpy emulator, rel err ~3.8e-3):
  q,k, cross q2/k2/v2, cproj:  1-pass fp8e4m3 DoubleRow (weights x128 on host)
  v, fc:                       3-pass DoubleRow at one PSUM scale:
                               W8*x8 + W8*dx8 + R8*x8  (~bf16 accuracy, 0.75x
                               bf16 PE cost); dx8 = fp8(x - x8) via subnormals
  aproj, mproj:                bf16 (their inputs o / u are produced bf16
                               directly, avoiding on-chip hi/lo splits)
Attention interior: q/k bf16, exp/P f32r, V-aug f32r with ones column for the
softmax denominator; causal masking via one precomputed [128,896] master mask.
Softmax exp runs on paired PSUM banks ([128,1024] per ACT op); masks, squares
and fp8 deltas run on GPSIMD to keep DVE available for PSUM-coupled work.
The residual stream lives in SBUF for the whole kernel.
"""

import sys
import numpy as np

sys.path.insert(0, "/opt/trn_rl_repo")

B, T, C = 8, 1024, 1024
H = 16
D = C // H          # 64
TI = 256
FF = 4 * C          # 4096
EPS = 1e-5
NCT = C // 128      # 8 c tiles
NTT = T // 128      # 8 t tiles
P = 128
WS = 128.0          # fp8 weight scale
WSI = 1.0 / WS

_CACHED = {}

# fp8 DR packs: [MC, 128, KK*2*Mc], elem [mc, p, (kk, ko, m)] =
# q8(WS*W)[256*kk + 128*ko + p, mc*Mc + m]; *R_p carry fp8(WS*W - deq(W8)).
WPACK = {
    "attn_p": (C, 3 * C, 256),
    "vR_p": (C, C, 256),
    "q_p": (C, C, 256),
    "k_p": (C, C, 256),
    "v2_p": (C, C, 256),
    "cproj_p": (C, C, 256),
    "fc_p": (C, FF, 256),
    "fcR_p": (C, FF, 256),
}
# bf16 stationary packs: [M//128, 128, (K//128)*128], elem [mc, p, (c, m)] =
# bf16(W)[128*c + p, 128*mc + m]
BPACK = {"aproj_b": (C, C), "mproj_b": (FF, C)}


def _build(flags):
    import concourse.tile as tile
    from concourse import bacc, mybir
    from concourse.masks import make_identity

    F32, F32R = mybir.dt.float32, mybir.dt.float32r
    BF16 = mybir.dt.bfloat16
    F8 = mybir.dt.float8e4
    AF = mybir.ActivationFunctionType
    OP = mybir.AluOpType
    DR = mybir.MatmulPerfMode.DoubleRow

    qk_bias, q2_bias, ab_bias, cp_bias, mp_bias = flags

    nc = bacc.Bacc("TRN2", target_bir_lowering=False, debug=False, num_devices=8)

    dr = {}
    dr["x"] = nc.dram_tensor("x", [T, C], F32, kind="ExternalInput")
    dr["x_img_feats"] = nc.dram_tensor("x_img_feats", [TI, C], F32, kind="ExternalInput")
    for nm, shp in [
        ("b_attn", [3 * C]), ("b_aproj", [C]),
        ("bq", [C]), ("bk", [C]), ("bv", [C]), ("bcproj", [C]),
        ("b_fc", [FF]), ("b_mproj", [C]),
    ]:
        dr[nm] = nc.dram_tensor(nm, shp, F32, kind="ExternalInput")
    for nm, (K, M, Mc) in WPACK.items():
        dr[nm] = nc.dram_tensor(nm, [M // Mc, P, (K // 256) * 2 * Mc], F8,
                                kind="ExternalInput")
    for nm, (K, M) in BPACK.items():
        dr[nm] = nc.dram_tensor(nm, [M // P, P, (K // P) * P], BF16,
                                kind="ExternalInput")
    out_d = nc.dram_tensor("out", [T, C], F32, kind="ExternalOutput")

    with tile.TileContext(nc) as tc, nc.allow_low_precision(
        reason="fp8 DoubleRow projections + bf16 attention are intentional"
    ):
        kw_cms = []

        def openp(**kw):
            cm = tc.tile_pool(**kw)
            return cm, cm.__enter__()

        def openkw(**kw):
            cm, p = openp(**kw)
            kw_cms.append(cm)
            return p

        # ---------------- kernel-wide pools (left-stack base) ----------------
        constp = openkw(name="const", bufs=1)
        fsrp = openkw(name="fsr", bufs=2)       # f32r [128,512] squares
        abp = openkw(name="ab", bufs=1)         # A_b [128,1024] + A_col
        rowp = openkw(name="rows", bufs=3)      # [1,1024] rows
        rbp = openkw(name="rb", bufs=4)         # [64,512] + [1,512] rden

        # ---------------- constants ----------------
        ident = constp.tile([P, P], F32)
        make_identity(nc, ident)
        identR = constp.tile([P, P], F32R)
        nc.vector.tensor_copy(out=identR, in_=ident)

        ones_col = constp.tile([P, 16], F32)
        nc.vector.memset(ones_col, 1.0)
        ones128R = constp.tile([P, 1], F32R)
        nc.vector.tensor_copy(out=ones128R, in_=ones_col[:, 0:1])
        o1x = constp.tile([1, P], F32)
        nc.vector.memset(o1x, 1.0)
        ones_1x128 = constp.tile([1, P], F32R)
        nc.vector.tensor_copy(out=ones_1x128, in_=o1x)
        epsS_t = constp.tile([1, 1], F32)
        nc.vector.memset(epsS_t, EPS * WS * WS)
        zeros384 = constp.tile([P, 384], F32)
        nc.vector.memset(zeros384, 0.0)

        master = constp.tile([P, 896], F32)
        nc.gpsimd.memset(master, 1.0)
        nc.gpsimd.affine_select(
            out=master, in_=master, compare_op=OP.is_ge, fill=0.0,
            base=-384, pattern=[[1, 896]], channel_multiplier=-1)

        # ================= P0: load & transpose x (issued first) =============
        res_cm, residp = openp(name="resid", bufs=NCT, side="right")
        resid = [residp.tile([P, T], F32R, tag="res", name="res") for _ in range(NCT)]

        tok_cm, tokp = openp(name="tok0", bufs=4)
        tp_cm, tpp = openp(name="psT0", bufs=2, space="PSUM")
        toks = []
        for tt in range(NTT):
            tok = tokp.tile([P, C], F32, tag="tok", name="tok")
            nc.sync.dma_start(out=tok, in_=dr["x"].ap()[tt * P:(tt + 1) * P, :])
            toks.append(tok)
        for tt in range(NTT):
            tok = toks[tt]
            for c in range(NCT):
                tps = tpp.tile([P, P], F32, tag="tp", name="tp")
                nc.tensor.transpose(tps, tok[:, c * P:(c + 1) * P], ident)
                if c % 2:
                    nc.vector.tensor_copy(out=resid[c][:, tt * P:(tt + 1) * P], in_=tps)
                else:
                    nc.scalar.copy(out=resid[c][:, tt * P:(tt + 1) * P], in_=tps)
        tp_cm.__exit__(None, None, None)
        tok_cm.__exit__(None, None, None)

        # ---------------- small input rows (issued after x) ----------------
        def load_cols(name, nf):
            t = constp.tile([P, nf], F32, name=name + "_c")
            nc.sync.dma_start(out=t, in_=dr[name].ap().rearrange("(f p) -> p f", p=P))
            return t

        bqk = constp.tile([P, 16], F32)
        nc.sync.dma_start(out=bqk, in_=dr["b_attn"].ap()[0:2 * C].rearrange("(f p) -> p f", p=P))
        bq_c = load_cols("bq", NCT)
        bk_c = load_cols("bk", NCT)
        bap_c = load_cols("b_aproj", NCT)
        bcp_c = load_cols("bcproj", NCT)
        bmp_c = load_cols("b_mproj", NCT)
        bfc_c = load_cols("b_fc", FF // 128)

        # ---------------- helpers ----------------
        def load_wp(name, mc, wpool):
            K, M, Mc = WPACK[name]
            KK = K // 256
            t = wpool.tile([P, KK, 2, Mc], F8, tag="wp", name="wp")
            nc.sync.dma_start(
                out=t,
                in_=dr[name].ap()[mc].rearrange("p (kk ko m) -> p kk ko m", kk=KK, ko=2))
            return t

        def load_wb(name, mc, wpool):
            K, M = BPACK[name]
            nk = K // P
            t = wpool.tile([P, nk, P], BF16, tag="wb", name="wb")
            src = dr[name].ap()[mc].rearrange("p (c m) -> p c m", m=P)
            nc.sync.dma_start(out=t[:, 0:nk // 2, :], in_=src[:, 0:nk // 2, :])
            nc.sync.dma_start(out=t[:, nk // 2:nk, :], in_=src[:, nk // 2:nk, :])
            return t

        def bcast_row(row_f32, dest_pool, psp, tag):
            rowr = rowp.tile([1, C], F32R, tag="row", name="rowr")
            nc.vector.tensor_copy(out=rowr, in_=row_f32)
            dest = dest_pool.tile([P, C], F32, tag=tag, name=tag)
            for cc in range(2):
                bps = psp.tile([P, 512], F32, tag="bc", name="bc")
                nc.tensor.matmul(bps, ones_1x128, rowr[:, 512 * cc:512 * (cc + 1)],
                                 start=True, stop=True)
                nc.scalar.copy(out=dest[:, 512 * cc:512 * (cc + 1)], in_=bps)
            return dest

        def ln_stats(xtiles, psp, with_col=False):
            """A_b [128,T] broadcast of A[t] = 1/(128*std[t]); opt A_col [128,NTT]."""
            sum_ps, sq_ps = [], []
            for tch in range(2):
                sp = psp.tile([1, 512], F32, tag="lnsum", name="lnsum")
                qp = psp.tile([1, 512], F32, tag="lnsq", name="lnsq")
                for c in range(NCT):
                    xs = xtiles[c][:, 512 * tch:512 * (tch + 1)]
                    nc.tensor.matmul(sp, ones128R, xs, start=(c == 0), stop=(c == NCT - 1))
                    sq = fsrp.tile([P, 512], F32R, tag="sq", name="sq")
                    nc.scalar.activation(out=sq, in_=xs, func=AF.Square, scale=1.0)
                    nc.tensor.matmul(qp, ones128R, sq, start=(c == 0), stop=(c == NCT - 1))
                sum_ps.append(sp)
                sq_ps.append(qp)
            mu = rowp.tile([1, T], F32, tag="row", name="mu")
            for tch in range(2):
                nc.vector.tensor_scalar_mul(out=mu[:, 512 * tch:512 * (tch + 1)],
                                            in0=sum_ps[tch], scalar1=1.0 / C)
            musq = rowp.tile([1, T], F32, tag="row", name="musq")
            nc.vector.tensor_tensor(out=musq, in0=mu, in1=mu, op=OP.mult)
            msq = rowp.tile([1, T], F32, tag="row", name="msq")
            for tch in range(2):
                sl = slice(512 * tch, 512 * (tch + 1))
                nc.vector.scalar_tensor_tensor(
                    out=msq[:, sl], in0=sq_ps[tch], scalar=1.0 / C,
                    in1=musq[:, sl], op0=OP.mult, op1=OP.subtract)
            nc.scalar.activation(out=musq, in_=msq, func=AF.Sqrt, bias=epsS_t,
                                 scale=WS * WS)
            arow = rowp.tile([1, T], F32R, tag="row", name="arow")
            nc.vector.reciprocal(out=arow, in_=musq)
            A_b = abp.tile([P, T], F32, tag="A_b", name="A_b")
            for tch in range(2):
                sl = slice(512 * tch, 512 * (tch + 1))
                bps = psp.tile([P, 512], F32, tag="bc", name="bc")
                nc.tensor.matmul(bps, ones_1x128, arow[:, sl], start=True, stop=True)
                nc.scalar.copy(out=A_b[:, sl], in_=bps)
            if not with_col:
                return A_b, None
            A_col = abp.tile([P, NTT], F32, tag="A_col", name="A_col")
            for tt in range(NTT):
                cps = psp.tile([P, P], F32, tag="bc", name="bc")
                nc.tensor.transpose(cps, A_b[:, tt * P:(tt + 1) * P], ident)
                nc.vector.tensor_copy(out=A_col[:, tt:tt + 1], in_=cps[:, 0:1])
            return A_b, A_col

        def quant_x(xtiles, x8, xd8, pool_only=False):
            """fp8 copy of the residual stream (+ optional fp8 delta)."""
            for tch in range(2):
                sl = slice(512 * tch, 512 * (tch + 1))
                for c in range(NCT):
                    eng = nc.gpsimd if (pool_only or c % 2 == 0) else nc.vector
                    eng.tensor_copy(out=x8[:, c, sl], in_=xtiles[c][:, sl])
            if xd8 is None:
                return
            for tch in range(2):
                sl = slice(512 * tch, 512 * (tch + 1))
                for c in range(NCT):
                    eng = nc.gpsimd if (pool_only or c % 2 == 0) else nc.vector
                    eng.scalar_tensor_tensor(
                        out=xd8[:, c, sl], in0=x8[:, c, sl], scalar=-1.0,
                        in1=xtiles[c][:, sl], op0=OP.mult, op1=OP.add)

        def attn_chunk(kq_of, vaug_tiles, n_s, h, tch, psp, ppool, causal,
                       o_all, rb_split=True, s_bufs=3, o_bufs=2):
            (kt, ko), (qt, qo) = kq_of(h)
            tsl = slice(512 * tch, 512 * (tch + 1))
            ptiles = []
            pair_ps = []
            for pr in range(n_s // 2):
                sps = psp.tile([P, 1024], F32, tag="s", name="s", bufs=s_bufs)
                for hf in range(2):
                    st = 2 * pr + hf
                    nc.tensor.matmul(sps[:, 512 * hf:512 * hf + 512],
                                     kt[ko:ko + D, st * P:(st + 1) * P],
                                     qt[qo:qo + D, tsl], start=True, stop=True,
                                     tile_position=(ko, 0))
                pair_ps.append(sps)
            for pr in range(n_s // 2):
                sps = pair_ps[pr]
                pt = ppool.tile([P, 1024], F32R, tag="p", name="p")
                j0 = 2 * pr - 4 * tch
                j1 = j0 + 1
                d0 = causal and j0 >= 0
                d1 = causal and j1 >= 0
                z0 = P * j0 if d0 else 0
                z1 = P * j1 if d1 else 0
                nc.scalar.activation(out=pt[:, z0:1024], in_=sps[:, z0:1024],
                                     func=AF.Exp, scale=0.125)
                if d0 and z0:
                    nc.gpsimd.tensor_copy(out=pt[:, 0:z0], in_=zeros384[:, 0:z0])
                if d1 and z1:
                    nc.gpsimd.tensor_copy(out=pt[:, 512:512 + z1], in_=zeros384[:, 0:z1])
                if d0:
                    nc.gpsimd.tensor_tensor(out=pt[:, z0:z0 + P], in0=pt[:, z0:z0 + P],
                                            in1=master[:, 384:512], op=OP.mult)
                if d1:
                    nc.gpsimd.tensor_tensor(out=pt[:, 512 + z1:512 + z1 + P],
                                            in0=pt[:, 512 + z1:512 + z1 + P],
                                            in1=master[:, 384:512], op=OP.mult)
                ptiles.append(pt)
            ops = psp.tile([P, 512], F32, tag="o", name="o", bufs=o_bufs)
            for st in range(n_s):
                pt = ptiles[st // 2][:, 512 * (st % 2):512 * (st % 2) + 512]
                nc.tensor.matmul(ops[0:65, :], vaug_tiles[st][:, 65 * h:65 * h + 65],
                                 pt, start=(st == 0), stop=(st == n_s - 1))
            rden = rbp.tile([1, 512], F32R, tag="rden", name="rden")
            nc.vector.reciprocal(out=rden, in_=ops[64:65, :])
            # broadcast 1/den into the unused partitions 64..127 of the same
            # PSUM tile (WAR on the den row orders this after the reciprocal)
            nc.tensor.matmul(ops[64:128, :], ones_1x128[:, 0:64], rden,
                             start=True, stop=True, tile_position=(0, 64))
            rb = rbp.tile([64, 512], F32, tag="rb", name="rb")
            if rb_split and h % 2:
                nc.vector.tensor_copy(out=rb, in_=ops[64:128, :])
            else:
                nc.scalar.copy(out=rb, in_=ops[64:128, :])
            po = (h % 2) * D
            nc.vector.tensor_tensor(out=o_all[po:po + D, h // 2, tsl],
                                    in0=ops[0:64, :], in1=rb, op=OP.mult)

        def dr_group(psum, pairs):
            n = len(pairs)
            for i, (lh, rh) in enumerate(pairs):
                nc.tensor.matmul(psum, lh, rh, start=(i == 0), stop=(i == n - 1),
                                 perf_mode=DR)

        def ws_passes(wt, wtR, h8, hd, msl, tsl2):
            ps = [(wt[:, kk, :, msl], h8[:, 2 * kk:2 * kk + 2, tsl2]) for kk in range(4)]
            if hd is not None:
                ps += [(wt[:, kk, :, msl], hd[:, 2 * kk:2 * kk + 2, tsl2]) for kk in range(4)]
            if wtR is not None:
                ps += [(wtR[:, kk, :, msl], h8[:, 2 * kk:2 * kk + 2, tsl2]) for kk in range(4)]
            return ps

        # ================= P1: LN1 + qkv projections =================
        x8_cm, x8p = openp(name="x8", bufs=1)
        x8 = x8p.tile([P, NCT, T], F8, tag="x8", name="x8")

        xd8_cm, xd8p = openp(name="xd8", bufs=1)
        xd8 = xd8p.tile([P, NCT, T], F8, tag="xd8", name="xd8")

        ln_cm, lnp = openp(name="psLN0", bufs=2, space="PSUM")
        A_b, A_col = ln_stats(resid, lnp, with_col=True)
        ln_cm.__exit__(None, None, None)
        quant_x(resid, x8, xd8)

        # ---- cross-attention K/V side (depends only on x_img_feats): hoisted
        # into the startup window so its latency chains overlap qkv compute.
        cross_cm, crossp = openp(name="cross", bufs=12, side="right")
        imgT = crossp.tile([P, NCT, TI], F8, tag="imgT", name="imgT", bufs=1)
        k2_t = [crossp.tile([P, TI], BF16, tag="k2", name="k2", bufs=NCT)
                for _ in range(NCT)]
        v2aug = [crossp.tile([P, 16 * 65], F32R, tag="va2", name="va2", bufs=2)
                 for _ in range(TI // P)]

        wk_cm, wk = openp(name="wk", bufs=3)
        tok_cm, tokp = openp(name="tok4", bufs=2)
        acckv_cm, acckv = openp(name="psKV", bufs=2, space="PSUM")
        tpi_cm, tpi = openp(name="psT4", bufs=2, space="PSUM")
        for tt in range(TI // P):
            tok = tokp.tile([P, C], F32, tag="tok", name="tok")
            nc.sync.dma_start(out=tok, in_=dr["x_img_feats"].ap()[tt * P:(tt + 1) * P, :])
            for c in range(NCT):
                tps = tpi.tile([P, P], F32, tag="tp", name="tp")
                nc.tensor.transpose(tps, tok[:, c * P:(c + 1) * P], ident)
                nc.vector.tensor_copy(out=imgT[:, c, tt * P:(tt + 1) * P], in_=tps)
        tpi_cm.__exit__(None, None, None)
        tok_cm.__exit__(None, None, None)

        for mc in range(4):
            wt = load_wp("k_p", mc, wk)
            for mh in range(2):
                f = 2 * mc + mh
                kps = acckv.tile([P, 256], F32, tag="acc256", name="acc256")
                dr_group(kps, [(wt[:, kk, :, 128 * mh:128 * mh + 128],
                                imgT[:, 2 * kk:2 * kk + 2, :]) for kk in range(4)])
                nc.scalar.activation(out=k2_t[f], in_=kps, func=AF.Identity,
                                     bias=bk_c[:, f:f + 1], scale=WSI)

        brow_v2 = rowp.tile([1, C], F32, tag="row", name="braw2")
        nc.sync.dma_start(out=brow_v2, in_=dr["bv"].ap().rearrange("(a c) -> a c", a=1))
        bvb2 = bcast_row(brow_v2, wk, acckv, "bvb2")
        for cc in range(4):
            wt = load_wp("v2_p", cc, wk)
            for st in range(TI // P):
                vps = acckv.tile([P, 256], F32, tag="acc256", name="acc256")
                dr_group(vps, [(imgT[:, 2 * kk:2 * kk + 2, st * P:(st + 1) * P],
                                wt[:, kk, :, :]) for kk in range(4)])
                dst = v2aug[st].rearrange("p (h x) -> p h x", x=65)[:, 4 * cc:4 * (cc + 1), 0:64]
                nc.vector.scalar_tensor_tensor(
                    out=dst, in0=vps.rearrange("p (h x) -> p h x", x=64),
                    scalar=WSI,
                    in1=bvb2[:, 256 * cc:256 * (cc + 1)].rearrange("p (h x) -> p h x", x=64),
                    op0=OP.mult, op1=OP.add)
        for st in range(TI // P):
            nc.vector.tensor_copy(
                out=v2aug[st].rearrange("p (h x) -> p h x", x=65)[:, :, 64:65],
                in_=ones_col.rearrange("p (h x) -> p h x", x=1))
        wk_cm.__exit__(None, None, None)

        # ---- self-attention v projection (3-pass) + q,k (1-pass)
        vap_cm, vap = openp(name="vaug", bufs=NTT, side="right")
        vaug = [vap.tile([P, 16 * 65], F32R, tag="va", name="va") for _ in range(NTT)]

        wv_cm, wv = openp(name="wv", bufs=4)
        accv_cm, accv = openp(name="psACv", bufs=2, space="PSUM")
        brow_v = rowp.tile([1, C], F32, tag="row", name="braw")
        nc.sync.dma_start(out=brow_v,
                          in_=dr["b_attn"].ap()[2 * C:3 * C].rearrange("(a c) -> a c", a=1))
        bvb1 = bcast_row(brow_v, wv, accv, "bvb")
        for cc in range(4):   # v output chunks of 256 cols (4 heads each)
            wt = load_wp("attn_p", 8 + cc, wv)
            wtR = load_wp("vR_p", cc, wv)
            for tt in range(NTT):
                vps = accv.tile([P, 256], F32, tag="acc", name="acc")
                tsl = slice(tt * P, (tt + 1) * P)
                ps = ([(x8[:, 2 * kk:2 * kk + 2, tsl], wt[:, kk, :, :]) for kk in range(4)]
                      + [(xd8[:, 2 * kk:2 * kk + 2, tsl], wt[:, kk, :, :]) for kk in range(4)]
                      + [(x8[:, 2 * kk:2 * kk + 2, tsl], wtR[:, kk, :, :]) for kk in range(4)])
                dr_group(vps, ps)
                dst = vaug[tt].rearrange("p (h x) -> p h x", x=65)[:, 4 * cc:4 * (cc + 1), 0:64]
                nc.vector.scalar_tensor_tensor(
                    out=dst, in0=vps.rearrange("p (h x) -> p h x", x=64),
                    scalar=A_col[:, tt:tt + 1],
                    in1=bvb1[:, 256 * cc:256 * (cc + 1)].rearrange("p (h x) -> p h x", x=64),
                    op0=OP.mult, op1=OP.add)
        for tt in range(NTT):
            nc.vector.tensor_copy(
                out=vaug[tt].rearrange("p (h x) -> p h x", x=65)[:, :, 64:65],
                in_=ones_col.rearrange("p (h x) -> p h x", x=1))
        accv_cm.__exit__(None, None, None)
        wv_cm.__exit__(None, None, None)
        acckv_cm.__exit__(None, None, None)
        xd8_cm.__exit__(None, None, None)

        qk_cm, qkp = openp(name="qk", bufs=16, side="right")
        w1_cm, w1 = openp(name="w1", bufs=3)
        acc_cm, accp = openp(name="psAC1", bufs=4, space="PSUM")
        qk_t = []
        for mc in range(8):
            wt = load_wp("attn_p", mc, w1)
            for mh in range(2):
                f = 2 * mc + mh
                qt = qkp.tile([P, T], BF16, tag="qk", name="qk")
                for tch in range(2):
                    sl = slice(512 * tch, 512 * (tch + 1))
                    aps = accp.tile([P, 512], F32, tag="acc", name="acc")
                    dr_group(aps, ws_passes(wt, None, x8, None,
                                            slice(128 * mh, 128 * mh + 128), sl))
                    nc.vector.tensor_tensor(out=qt[:, sl], in0=aps, in1=A_b[:, sl],
                                            op=OP.mult)
                    if qk_bias:
                        nc.vector.tensor_scalar_add(out=qt[:, sl], in0=qt[:, sl],
                                                    scalar1=bqk[:, f:f + 1])
                qk_t.append(qt)
        acc_cm.__exit__(None, None, None)
        w1_cm.__exit__(None, None, None)
        x8_cm.__exit__(None, None, None)

        # ================= P2: self attention =================
        o_cm, opool = openp(name="o1", bufs=1)
        o_all = opool.tile([P, NCT, T], BF16, tag="ot", name="ot")
        pp_cm, pp = openp(name="pp1", bufs=5)
        psS_cm, psS = openp(name="psS1", bufs=2, space="PSUM")

        def kq_self(h):
            return (qk_t[8 + h // 2], (h % 2) * D), (qk_t[h // 2], (h % 2) * D)

        for tch in range(2):
            for h in range(H):
                attn_chunk(kq_self, vaug, 4 * (tch + 1), h, tch, psS, pp,
                           causal=True, o_all=o_all)

        psS_cm.__exit__(None, None, None)
        pp_cm.__exit__(None, None, None)
        qk_cm.__exit__(None, None, None)
        vap_cm.__exit__(None, None, None)

        # ================= P3: aproj (bf16) + residual in place ======
        w2_cm, w2 = openp(name="w2", bufs=3)
        acc_cm, accp = openp(name="psAC3", bufs=3, space="PSUM")
        for co in range(NCT):
            wt = load_wb("aproj_b", co, w2)
            for tch in range(2):
                sl = slice(512 * tch, 512 * (tch + 1))
                aps = accp.tile([P, 512], F32, tag="acc", name="acc")
                for c in range(NCT):
                    nc.tensor.matmul(aps, wt[:, c, :], o_all[:, c, sl],
                                     start=(c == 0), stop=(c == NCT - 1))
                nc.vector.tensor_tensor(out=resid[co][:, sl], in0=aps,
                                        in1=resid[co][:, sl], op=OP.add)
                if ab_bias:
                    nc.vector.tensor_scalar_add(
                        out=resid[co][:, sl], in0=resid[co][:, sl],
                        scalar1=bap_c[:, co:co + 1])
        acc_cm.__exit__(None, None, None)
        w2_cm.__exit__(None, None, None)
        o_cm.__exit__(None, None, None)

        # ================= P4: cross attention projections =================
        x1_cm, x1p = openp(name="x18", bufs=1)
        x18 = x1p.tile([P, NCT, T], F8, tag="x8", name="x8")

        ln_cm, lnp = openp(name="psLN1", bufs=2, space="PSUM")
        A_b, _ = ln_stats(resid, lnp)
        ln_cm.__exit__(None, None, None)
        quant_x(resid, x18, None, pool_only=True)

        w3_cm, w3 = openp(name="w3", bufs=3)
        acc_cm, accp = openp(name="psAC4", bufs=4, space="PSUM")
        q2_cm, q2p = openp(name="q2", bufs=NCT, side="right")
        q2_t = []
        for mc in range(4):
            wt = load_wp("q_p", mc, w3)
            for mh in range(2):
                f = 2 * mc + mh
                qt = q2p.tile([P, T], BF16, tag="q2", name="q2")
                for tch in range(2):
                    sl = slice(512 * tch, 512 * (tch + 1))
                    aps = accp.tile([P, 512], F32, tag="acc", name="acc")
                    dr_group(aps, ws_passes(wt, None, x18, None,
                                            slice(128 * mh, 128 * mh + 128), sl))
                    nc.vector.tensor_tensor(out=qt[:, sl], in0=aps, in1=A_b[:, sl],
                                            op=OP.mult)
                    if q2_bias:
                        nc.vector.tensor_scalar_add(out=qt[:, sl], in0=qt[:, sl],
                                                    scalar1=bq_c[:, f:f + 1])
                q2_t.append(qt)
        acc_cm.__exit__(None, None, None)
        w3_cm.__exit__(None, None, None)
        x1_cm.__exit__(None, None, None)

        # ================= P5: cross attention =================
        o_cm, opool = openp(name="o2", bufs=1)
        o2_all = opool.tile([P, NCT, T], F8, tag="ot", name="ot")
        pp_cm, pp = openp(name="pp2", bufs=4)
        psS_cm, psS = openp(name="psS2", bufs=2, space="PSUM")

        def kq_cross(h):
            return (k2_t[h // 2], (h % 2) * D), (q2_t[h // 2], (h % 2) * D)

        for tch in range(2):
            for h in range(H):
                attn_chunk(kq_cross, v2aug, TI // P, h, tch, psS, pp,
                           causal=False, o_all=o2_all, rb_split=False,
                           s_bufs=2, o_bufs=4)

        psS_cm.__exit__(None, None, None)
        pp_cm.__exit__(None, None, None)
        q2_cm.__exit__(None, None, None)
        cross_cm.__exit__(None, None, None)

        # ================= P6: cproj + residual (x2, in place) =================
        w4_cm, w4 = openp(name="w4", bufs=3)
        acc_cm, accp = openp(name="psAC5", bufs=3, space="PSUM")
        for mc in range(4):
            wt = load_wp("cproj_p", mc, w4)
            for mh in range(2):
                co = 2 * mc + mh
                for tch in range(2):
                    sl = slice(512 * tch, 512 * (tch + 1))
                    aps = accp.tile([P, 512], F32, tag="acc", name="acc")
                    dr_group(aps, ws_passes(wt, None, o2_all, None,
                                            slice(128 * mh, 128 * mh + 128), sl))
                    nc.vector.scalar_tensor_tensor(
                        out=resid[co][:, sl], in0=aps, scalar=WSI,
                        in1=resid[co][:, sl], op0=OP.mult, op1=OP.add)
                    if cp_bias:
                        nc.vector.tensor_scalar_add(
                            out=resid[co][:, sl], in0=resid[co][:, sl],
                            scalar1=bcp_c[:, co:co + 1])
        acc_cm.__exit__(None, None, None)
        w4_cm.__exit__(None, None, None)
        o_cm.__exit__(None, None, None)

        # ================= P7: MLP =================
        x2_cm, x2p = openp(name="x28", bufs=1)
        x28 = x2p.tile([P, NCT, T], F8, tag="x8", name="x8")
        x2d8 = x2p.tile([P, NCT, T], F8, tag="xd8", name="xd8")

        ln_cm, lnp = openp(name="psLN2", bufs=2, space="PSUM")
        A_b, _ = ln_stats(resid, lnp)
        ln_cm.__exit__(None, None, None)
        quant_x(resid, x28, x2d8)

        up_cm, up = openp(name="u", bufs=16, side="right")
        utiles = [up.tile([P, 2, T], BF16, tag="u", name="u") for _ in range(16)]
        uscr_cm, uscrp = openp(name="uscr", bufs=4)
        w5_cm, w5 = openp(name="w5", bufs=4)
        accU_cm, accU = openp(name="psU", bufs=4, space="PSUM")
        for mc in range(16):
            wt = load_wp("fc_p", mc, w5)
            wtR = load_wp("fcR_p", mc, w5)
            for mh in range(2):
                ff = 2 * mc + mh
                for tch in range(2):
                    sl = slice(512 * tch, 512 * (tch + 1))
                    ups = accU.tile([P, 512], F32, tag="acc", name="acc")
                    dr_group(ups, ws_passes(wt, wtR, x28, x2d8,
                                            slice(128 * mh, 128 * mh + 128), sl))
                    uscr = uscrp.tile([P, 512], F32, tag="us", name="us")
                    nc.vector.tensor_tensor(out=uscr, in0=ups, in1=A_b[:, sl],
                                            op=OP.mult)
                    nc.scalar.activation(out=utiles[ff // 2][:, ff % 2, sl], in_=uscr,
                                         func=AF.Gelu_apprx_tanh,
                                         bias=bfc_c[:, ff:ff + 1], scale=1.0)
        accU_cm.__exit__(None, None, None)
        w5_cm.__exit__(None, None, None)
        uscr_cm.__exit__(None, None, None)
        x2_cm.__exit__(None, None, None)

        w6_cm, w6 = openp(name="w6", bufs=3)
        psM_cm, psM = openp(name="psM", bufs=3, space="PSUM")
        for co in range(NCT):
            wt = load_wb("mproj_b", co, w6)
            for tch in range(2):
                sl = slice(512 * tch, 512 * (tch + 1))
                mps = psM.tile([P, 512], F32, tag="m", name="m")
                for ff in range(FF // P):
                    nc.tensor.matmul(mps, wt[:, ff, :], utiles[ff // 2][:, ff % 2, sl],
                                     start=(ff == 0), stop=(ff == FF // P - 1))
                nc.vector.tensor_tensor(out=resid[co][:, sl], in0=mps,
                                        in1=resid[co][:, sl], op=OP.add)
                if mp_bias:
                    nc.vector.tensor_scalar_add(
                        out=resid[co][:, sl], in0=resid[co][:, sl],
                        scalar1=bmp_c[:, co:co + 1])
        psM_cm.__exit__(None, None, None)
        w6_cm.__exit__(None, None, None)
        up_cm.__exit__(None, None, None)

        # ================= P8: transpose back & store =================
        tok_cm, tokp = openp(name="tok7", bufs=2)
        tp_cm, tpp = openp(name="psT7", bufs=4, space="PSUM")
        for tt in range(NTT):
            otok = tokp.tile([P, C], F32, tag="tok", name="tok")
            for c in range(NCT):
                tps = tpp.tile([P, P], F32R, tag="tpr", name="tpr")
                nc.tensor.transpose(tps, resid[c][:, tt * P:(tt + 1) * P], identR)
                if c % 2:
                    nc.vector.tensor_copy(out=otok[:, c * P:(c + 1) * P], in_=tps)
                else:
                    nc.scalar.copy(out=otok[:, c * P:(c + 1) * P], in_=tps)
            nc.sync.dma_start(out=out_d.ap()[tt * P:(tt + 1) * P, :], in_=otok)
        tp_cm.__exit__(None, None, None)
        tok_cm.__exit__(None, None, None)
        res_cm.__exit__(None, None, None)

        for cm in reversed(kw_cms):
            cm.__exit__(None, None, None)

    nc.compile()
    return nc


def _pack_core(Wq, Mc):
    K, M = Wq.shape
    KK, MC = K // 256, M // Mc
    A = Wq.reshape(KK, 2, P, MC, Mc).transpose(3, 2, 0, 1, 4)
    return np.ascontiguousarray(A.reshape(MC, P, KK * 2 * Mc))


def _pack_w(W, Mc=256):
    import ml_dtypes
    return _pack_core((np.asarray(W, np.float32) * WS).astype(ml_dtypes.float8_e4m3), Mc)


def _pack_wr(W, Mc=256):
    import ml_dtypes
    Ws = np.asarray(W, np.float32) * WS
    W8 = Ws.astype(ml_dtypes.float8_e4m3)
    R = Ws - W8.astype(np.float32)
    return _pack_core(R.astype(ml_dtypes.float8_e4m3), Mc)


def _pack_b(W):
    """[K, M] f32 -> [M//128, 128, (K//128)*128] bf16 stationary slabs."""
    import ml_dtypes
    K, M = W.shape
    A = np.asarray(W, np.float32).reshape(K // P, P, M // P, P).transpose(2, 1, 0, 3)
    return np.ascontiguousarray(A.reshape(M // P, P, K).astype(ml_dtypes.bfloat16))


def _fold(W, g):
    Wg = np.asarray(W, np.float32) * np.asarray(g, np.float32)[:, None]
    return Wg - Wg.mean(0, keepdims=True)


def kernel(**inputs):
    from concourse.bass_utils import run_bass_kernel_spmd

    np_inputs = {k: np.asarray(v, dtype=np.float32) for k, v in inputs.items()}
    g1, b1 = np_inputs["ln1_g"], np_inputs["ln1_b"]
    g2, b2 = np_inputs["ln2_g"], np_inputs["ln2_b"]
    Wa = np_inputs["W_attn"]

    # fold LN gains/means into weights; fold LN biases into effective biases
    Wa_f = _fold(Wa, g1)
    Wq_f = _fold(np_inputs["Wq"], g1)
    Wfc_f = _fold(np_inputs["W_fc"], g2)
    b_attn_eff = np_inputs["b_attn"] + Wa.T @ b1
    bq_eff = np_inputs["bq"] + np_inputs["Wq"].T @ b1
    bfc_eff = np_inputs["b_fc"] + np_inputs["W_fc"].T @ b2

    flags = (bool(np.any(b_attn_eff[0:2 * C])), bool(np.any(bq_eff)),
             bool(np.any(np_inputs["b_aproj"])), bool(np.any(np_inputs["bcproj"])),
             bool(np.any(np_inputs["b_mproj"])))
    key = ("nc", flags)
    if key not in _CACHED:
        _CACHED[key] = _build(flags)
    nc = _CACHED[key]

    packed = {
        "attn_p": _pack_w(Wa_f),
        "vR_p": _pack_wr(Wa_f[:, 2 * C:3 * C]),
        "q_p": _pack_w(Wq_f),
        "k_p": _pack_w(np_inputs["Wk"]),
        "v2_p": _pack_w(np_inputs["Wv"]),
        "cproj_p": _pack_w(np_inputs["Wcproj"]),
        "fc_p": _pack_w(Wfc_f),
        "fcR_p": _pack_wr(Wfc_f),
        "aproj_b": _pack_b(np_inputs["W_aproj"]),
        "mproj_b": _pack_b(np_inputs["W_mproj"]),
    }
    small = {
        "b_attn": b_attn_eff, "bq": bq_eff, "b_fc": bfc_eff,
        "b_aproj": np_inputs["b_aproj"], "bk": np_inputs["bk"],
        "bv": np_inputs["bv"], "bcproj": np_inputs["bcproj"],
        "b_mproj": np_inputs["b_mproj"],
    }
    in_maps = []
    for b in range(B):
        m = dict(small)
        m.update(packed)
        m["x"] = np.ascontiguousarray(np_inputs["x"][b])
        m["x_img_feats"] = np.ascontiguousarray(np_inputs["x_img_feats"][b])
        in_maps.append(m)
    res = run_bass_kernel_spmd(nc, in_maps, core_ids=list(range(B)))
    out = np.stack([res.results[b]["out"] for b in range(B)], axis=0)
    return out.astype(np.float32)


# revision 51
# speedup vs baseline: 1.0221x; 1.0028x over previous
"""Trainium2 Bass kernel for a dense transformer block (self-attn + cross-attn + MLP).

Sharding: data-parallel over batch, one batch element per NeuronCore (B=8, 8 cores),
no collectives. Activations are feature-major ([C, T]) on chip.

LayerNorm is FOLDED into the projection weights on the host:
    W' = g*W - colmean(g*W),  bias' = b + W^T ln_b
so projections consume the RAW residual x (quantized straight off the stream,
no LN-apply pass); the per-token scale A[t] = 1/(128*std[t]) is applied at PSUM
evacuation (a [128,T] broadcast tile built once per LN via a ones-matmul).

Precision plan (validated in a calibrated numpy emulator, rel err ~3.8e-3):
  q,k, cross q2/k2/v2, cproj:  1-pass fp8e4m3 DoubleRow (weights x128 on host)
  v, fc:                       3-pass DoubleRow at one PSUM scale:
                               W8*x8 + W8*dx8 + R8*x8  (~bf16 accuracy, 0.75x
                               bf16 PE cost); dx8 = fp8(x - x8) via subnormals
  aproj, mproj:                bf16 (their inputs o / u are produced bf16
                               directly, avoiding on-chip hi/lo splits)
Attention interior: q/k bf16, exp/P f32r, V-aug f32r with ones column for the
softmax denominator; causal masking via one precomputed [128,896] master mask.
Softmax exp runs on paired PSUM banks ([128,1024] per ACT op); masks, squares
and fp8 deltas run on GPSIMD to keep DVE available for PSUM-coupled work.
The residual stream lives in SBUF for the whole kernel.
"""

import sys
import numpy as np

sys.path.insert(0, "/opt/trn_rl_repo")

B, T, C = 8, 1024, 1024
H = 16
D = C // H          # 64
TI = 256
FF = 4 * C          # 4096
EPS = 1e-5
NCT = C // 128      # 8 c tiles
NTT = T // 128      # 8 t tiles
P = 128
WS = 128.0          # fp8 weight scale
WSI = 1.0 / WS

_CACHED = {}

# fp8 DR packs: [MC, 128, KK*2*Mc], elem [mc, p, (kk, ko, m)] =
# q8(WS*W)[256*kk + 128*ko + p, mc*Mc + m]; *R_p carry fp8(WS*W - deq(W8)).
WPACK = {
    "attn_p": (C, 3 * C, 256),
    "vR_p": (C, C, 256),
    "q_p": (C, C, 256),
    "k_p": (C, C, 256),
    "v2_p": (C, C, 256),
    "cproj_p": (C, C, 256),
    "fc_p": (C, FF, 256),
    "fcR_p": (C, FF, 256),
}
# bf16 stationary packs: [M//128, 128, (K//128)*128], elem [mc, p, (c, m)] =
# bf16(W)[128*c + p, 128*mc + m]
BPACK = {"aproj_b": (C, C), "mproj_b": (FF, C)}


def _build(flags):
    import concourse.tile as tile
    from concourse import bacc, mybir
    from concourse.masks import make_identity

    F32, F32R = mybir.dt.float32, mybir.dt.float32r
    BF16 = mybir.dt.bfloat16
    F8 = mybir.dt.float8e4
    AF = mybir.ActivationFunctionType
    OP = mybir.AluOpType
    DR = mybir.MatmulPerfMode.DoubleRow

    qk_bias, q2_bias, ab_bias, cp_bias, mp_bias = flags

    nc = bacc.Bacc("TRN2", target_bir_lowering=False, debug=False, num_devices=8)

    dr = {}
    dr["x"] = nc.dram_tensor("x", [T, C], F32, kind="ExternalInput")
    dr["x_img_feats"] = nc.dram_tensor("x_img_feats", [TI, C], F32, kind="ExternalInput")
    for nm, shp in [
        ("b_attn", [3 * C]), ("b_aproj", [C]),
        ("bq", [C]), ("bk", [C]), ("bv", [C]), ("bcproj", [C]),
        ("b_fc", [FF]), ("b_mproj", [C]),
    ]:
        dr[nm] = nc.dram_tensor(nm, shp, F32, kind="ExternalInput")
    for nm, (K, M, Mc) in WPACK.items():
        dr[nm] = nc.dram_tensor(nm, [M // Mc, P, (K // 256) * 2 * Mc], F8,
                                kind="ExternalInput")
    for nm, (K, M) in BPACK.items():
        dr[nm] = nc.dram_tensor(nm, [M // P, P, (K // P) * P], BF16,
                                kind="ExternalInput")
    out_d = nc.dram_tensor("out", [T, C], F32, kind="ExternalOutput")

    with tile.TileContext(nc) as tc, nc.allow_low_precision(
        reason="fp8 DoubleRow projections + bf16 attention are intentional"
    ):
        kw_cms = []

        def openp(**kw):
            cm = tc.tile_pool(**kw)
            return cm, cm.__enter__()

        def openkw(**kw):
            cm, p = openp(**kw)
            kw_cms.append(cm)
            return p

        # ---------------- kernel-wide pools (left-stack base) ----------------
        constp = openkw(name="const", bufs=1)
        fsrp = openkw(name="fsr", bufs=2)       # f32r [128,512] squares
        abp = openkw(name="ab", bufs=1)         # A_b [128,1024] + A_col
        rowp = openkw(name="rows", bufs=3)      # [1,1024] rows
        rbp = openkw(name="rb", bufs=4)         # [64,512] + [1,512] rden

        # ---------------- constants ----------------
        ident = constp.tile([P, P], F32)
        make_identity(nc, ident)
        identR = constp.tile([P, P], F32R)
        nc.vector.tensor_copy(out=identR, in_=ident)

        ones_col = constp.tile([P, 16], F32)
        nc.vector.memset(ones_col, 1.0)
        ones128R = constp.tile([P, 1], F32R)
        nc.vector.tensor_copy(out=ones128R, in_=ones_col[:, 0:1])
        o1x = constp.tile([1, P], F32)
        nc.vector.memset(o1x, 1.0)
        ones_1x128 = constp.tile([1, P], F32R)
        nc.vector.tensor_copy(out=ones_1x128, in_=o1x)
        epsS_t = constp.tile([1, 1], F32)
        nc.vector.memset(epsS_t, EPS * WS * WS)
        zeros384 = constp.tile([P, 384], F32)
        nc.vector.memset(zeros384, 0.0)

        master = constp.tile([P, 896], F32)
        nc.gpsimd.memset(master, 1.0)
        nc.gpsimd.affine_select(
            out=master, in_=master, compare_op=OP.is_ge, fill=0.0,
            base=-384, pattern=[[1, 896]], channel_multiplier=-1)

        # ================= P0: load & transpose x (issued first) =============
        res_cm, residp = openp(name="resid", bufs=NCT, side="right")
        resid = [residp.tile([P, T], F32R, tag="res", name="res") for _ in range(NCT)]

        tok_cm, tokp = openp(name="tok0", bufs=4)
        tp_cm, tpp = openp(name="psT0", bufs=2, space="PSUM")
        toks = []
        for tt in range(NTT):
            tok = tokp.tile([P, C], F32, tag="tok", name="tok")
            for hf in range(2):
                nc.sync.dma_start(out=tok[:, 512 * hf:512 * (hf + 1)],
                                  in_=dr["x"].ap()[tt * P:(tt + 1) * P,
                                                   512 * hf:512 * (hf + 1)])
            toks.append(tok)
        for tt in range(NTT):
            tok = toks[tt]
            for c in range(NCT):
                tps = tpp.tile([P, P], F32, tag="tp", name="tp")
                nc.tensor.transpose(tps, tok[:, c * P:(c + 1) * P], ident)
                if c % 2:
                    nc.vector.tensor_copy(out=resid[c][:, tt * P:(tt + 1) * P], in_=tps)
                else:
                    nc.scalar.copy(out=resid[c][:, tt * P:(tt + 1) * P], in_=tps)
        tp_cm.__exit__(None, None, None)
        tok_cm.__exit__(None, None, None)

        # ---------------- small input rows (issued after x) ----------------
        def load_cols(name, nf):
            t = constp.tile([P, nf], F32, name=name + "_c")
            nc.sync.dma_start(out=t, in_=dr[name].ap().rearrange("(f p) -> p f", p=P))
            return t

        bqk = constp.tile([P, 16], F32)
        nc.sync.dma_start(out=bqk, in_=dr["b_attn"].ap()[0:2 * C].rearrange("(f p) -> p f", p=P))
        bq_c = load_cols("bq", NCT)
        bk_c = load_cols("bk", NCT)
        bap_c = load_cols("b_aproj", NCT)
        bcp_c = load_cols("bcproj", NCT)
        bmp_c = load_cols("b_mproj", NCT)
        bfc_c = load_cols("b_fc", FF // 128)

        # ---------------- helpers ----------------
        def load_wp(name, mc, wpool):
            K, M, Mc = WPACK[name]
            KK = K // 256
            t = wpool.tile([P, KK, 2, Mc], F8, tag="wp", name="wp")
            nc.sync.dma_start(
                out=t,
                in_=dr[name].ap()[mc].rearrange("p (kk ko m) -> p kk ko m", kk=KK, ko=2))
            return t

        def load_wb(name, mc, wpool):
            K, M = BPACK[name]
            nk = K // P
            t = wpool.tile([P, nk, P], BF16, tag="wb", name="wb")
            src = dr[name].ap()[mc].rearrange("p (c m) -> p c m", m=P)
            nc.sync.dma_start(out=t[:, 0:nk // 2, :], in_=src[:, 0:nk // 2, :])
            nc.sync.dma_start(out=t[:, nk // 2:nk, :], in_=src[:, nk // 2:nk, :])
            return t

        def bcast_row(row_f32, dest_pool, psp, tag):
            rowr = rowp.tile([1, C], F32R, tag="row", name="rowr")
            nc.vector.tensor_copy(out=rowr, in_=row_f32)
            dest = dest_pool.tile([P, C], F32, tag=tag, name=tag)
            for cc in range(2):
                bps = psp.tile([P, 512], F32, tag="bc", name="bc")
                nc.tensor.matmul(bps, ones_1x128, rowr[:, 512 * cc:512 * (cc + 1)],
                                 start=True, stop=True)
                nc.scalar.copy(out=dest[:, 512 * cc:512 * (cc + 1)], in_=bps)
            return dest

        def ln_stats(xtiles, psp, with_col=False):
            """A_b [128,T] broadcast of A[t] = 1/(128*std[t]); opt A_col [128,NTT]."""
            sum_ps, sq_ps = [], []
            for tch in range(2):
                sp = psp.tile([1, 512], F32, tag="lnsum", name="lnsum")
                qp = psp.tile([1, 512], F32, tag="lnsq", name="lnsq")
                for c in range(NCT):
                    xs = xtiles[c][:, 512 * tch:512 * (tch + 1)]
                    nc.tensor.matmul(sp, ones128R, xs, start=(c == 0), stop=(c == NCT - 1))
                    sq = fsrp.tile([P, 512], F32R, tag="sq", name="sq")
                    nc.scalar.activation(out=sq, in_=xs, func=AF.Square, scale=1.0)
                    nc.tensor.matmul(qp, ones128R, sq, start=(c == 0), stop=(c == NCT - 1))
                sum_ps.append(sp)
                sq_ps.append(qp)
            mu = rowp.tile([1, T], F32, tag="row", name="mu")
            for tch in range(2):
                nc.vector.tensor_scalar_mul(out=mu[:, 512 * tch:512 * (tch + 1)],
                                            in0=sum_ps[tch], scalar1=1.0 / C)
            musq = rowp.tile([1, T], F32, tag="row", name="musq")
            nc.vector.tensor_tensor(out=musq, in0=mu, in1=mu, op=OP.mult)
            msq = rowp.tile([1, T], F32, tag="row", name="msq")
            for tch in range(2):
                sl = slice(512 * tch, 512 * (tch + 1))
                nc.vector.scalar_tensor_tensor(
                    out=msq[:, sl], in0=sq_ps[tch], scalar=1.0 / C,
                    in1=musq[:, sl], op0=OP.mult, op1=OP.subtract)
            nc.scalar.activation(out=musq, in_=msq, func=AF.Sqrt, bias=epsS_t,
                                 scale=WS * WS)
            arow = rowp.tile([1, T], F32R, tag="row", name="arow")
            nc.vector.reciprocal(out=arow, in_=musq)
            A_b = abp.tile([P, T], F32, tag="A_b", name="A_b")
            for tch in range(2):
                sl = slice(512 * tch, 512 * (tch + 1))
                bps = psp.tile([P, 512], F32, tag="bc", name="bc")
                nc.tensor.matmul(bps, ones_1x128, arow[:, sl], start=True, stop=True)
                nc.scalar.copy(out=A_b[:, sl], in_=bps)
            if not with_col:
                return A_b, None
            A_col = abp.tile([P, NTT], F32, tag="A_col", name="A_col")
            for tt in range(NTT):
                cps = psp.tile([P, P], F32, tag="bc", name="bc")
                nc.tensor.transpose(cps, A_b[:, tt * P:(tt + 1) * P], ident)
                nc.vector.tensor_copy(out=A_col[:, tt:tt + 1], in_=cps[:, 0:1])
            return A_b, A_col

        def quant_x(xtiles, x8, xd8, pool_only=False):
            """fp8 copy of the residual stream (+ optional fp8 delta)."""
            for tch in range(2):
                sl = slice(512 * tch, 512 * (tch + 1))
                for c in range(NCT):
                    eng = nc.gpsimd if (pool_only or c % 2 == 0) else nc.vector
                    eng.tensor_copy(out=x8[:, c, sl], in_=xtiles[c][:, sl])
            if xd8 is None:
                return
            for tch in range(2):
                sl = slice(512 * tch, 512 * (tch + 1))
                for c in range(NCT):
                    eng = nc.gpsimd if (pool_only or c % 2 == 0) else nc.vector
                    eng.scalar_tensor_tensor(
                        out=xd8[:, c, sl], in0=x8[:, c, sl], scalar=-1.0,
                        in1=xtiles[c][:, sl], op0=OP.mult, op1=OP.add)

        def attn_chunk(kq_of, vaug_tiles, n_s, h, tch, psp, ppool, causal,
                       o_all, rb_split=True, s_bufs=3, o_bufs=2):
            (kt, ko), (qt, qo) = kq_of(h)
            tsl = slice(512 * tch, 512 * (tch + 1))
            ptiles = []
            pair_ps = []
            for pr in range(n_s // 2):
                sps = psp.tile([P, 1024], F32, tag="s", name="s", bufs=s_bufs)
                for hf in range(2):
                    st = 2 * pr + hf
                    nc.tensor.matmul(sps[:, 512 * hf:512 * hf + 512],
                                     kt[ko:ko + D, st * P:(st + 1) * P],
                                     qt[qo:qo + D, tsl], start=True, stop=True,
                                     tile_position=(ko, 0))
                pair_ps.append(sps)
            for pr in range(n_s // 2):
                sps = pair_ps[pr]
                pt = ppool.tile([P, 1024], F32R, tag="p", name="p")
                j0 = 2 * pr - 4 * tch
                j1 = j0 + 1
                d0 = causal and j0 >= 0
                d1 = causal and j1 >= 0
                z0 = P * j0 if d0 else 0
                z1 = P * j1 if d1 else 0
                nc.scalar.activation(out=pt[:, z0:1024], in_=sps[:, z0:1024],
                                     func=AF.Exp, scale=0.125)
                if d0 and z0:
                    nc.gpsimd.tensor_copy(out=pt[:, 0:z0], in_=zeros384[:, 0:z0])
                if d1 and z1:
                    nc.gpsimd.tensor_copy(out=pt[:, 512:512 + z1], in_=zeros384[:, 0:z1])
                if d0:
                    nc.gpsimd.tensor_tensor(out=pt[:, z0:z0 + P], in0=pt[:, z0:z0 + P],
                                            in1=master[:, 384:512], op=OP.mult)
                if d1:
                    nc.gpsimd.tensor_tensor(out=pt[:, 512 + z1:512 + z1 + P],
                                            in0=pt[:, 512 + z1:512 + z1 + P],
                                            in1=master[:, 384:512], op=OP.mult)
                ptiles.append(pt)
            ops = psp.tile([P, 512], F32, tag="o", name="o", bufs=o_bufs)
            for st in range(n_s):
                pt = ptiles[st // 2][:, 512 * (st % 2):512 * (st % 2) + 512]
                nc.tensor.matmul(ops[0:65, :], vaug_tiles[st][:, 65 * h:65 * h + 65],
                                 pt, start=(st == 0), stop=(st == n_s - 1))
            rden = rbp.tile([1, 512], F32R, tag="rden", name="rden")
            nc.vector.reciprocal(out=rden, in_=ops[64:65, :])
            # broadcast 1/den into the unused partitions 64..127 of the same
            # PSUM tile (WAR on the den row orders this after the reciprocal)
            nc.tensor.matmul(ops[64:128, :], ones_1x128[:, 0:64], rden,
                             start=True, stop=True, tile_position=(0, 64))
            rb = rbp.tile([64, 512], F32, tag="rb", name="rb")
            if rb_split and h % 2:
                nc.vector.tensor_copy(out=rb, in_=ops[64:128, :])
            else:
                nc.scalar.copy(out=rb, in_=ops[64:128, :])
            po = (h % 2) * D
            nc.vector.tensor_tensor(out=o_all[po:po + D, h // 2, tsl],
                                    in0=ops[0:64, :], in1=rb, op=OP.mult)

        def dr_group(psum, pairs):
            n = len(pairs)
            for i, (lh, rh) in enumerate(pairs):
                nc.tensor.matmul(psum, lh, rh, start=(i == 0), stop=(i == n - 1),
                                 perf_mode=DR)

        def ws_passes(wt, wtR, h8, hd, msl, tsl2):
            ps = [(wt[:, kk, :, msl], h8[:, 2 * kk:2 * kk + 2, tsl2]) for kk in range(4)]
            if hd is not None:
                ps += [(wt[:, kk, :, msl], hd[:, 2 * kk:2 * kk + 2, tsl2]) for kk in range(4)]
            if wtR is not None:
                ps += [(wtR[:, kk, :, msl], h8[:, 2 * kk:2 * kk + 2, tsl2]) for kk in range(4)]
            return ps

        # ================= P1: LN1 + qkv projections =================
        x8_cm, x8p = openp(name="x8", bufs=1)
        x8 = x8p.tile([P, NCT, T], F8, tag="x8", name="x8")

        xd8_cm, xd8p = openp(name="xd8", bufs=1)
        xd8 = xd8p.tile([P, NCT, T], F8, tag="xd8", name="xd8")

        ln_cm, lnp = openp(name="psLN0", bufs=2, space="PSUM")
        A_b, A_col = ln_stats(resid, lnp, with_col=True)
        ln_cm.__exit__(None, None, None)
        quant_x(resid, x8, xd8)

        # ---- cross-attention K/V side (depends only on x_img_feats): hoisted
        # into the startup window so its latency chains overlap qkv compute.
        cross_cm, crossp = openp(name="cross", bufs=12, side="right")
        imgT = crossp.tile([P, NCT, TI], F8, tag="imgT", name="imgT", bufs=1)
        k2_t = [crossp.tile([P, TI], BF16, tag="k2", name="k2", bufs=NCT)
                for _ in range(NCT)]
        v2aug = [crossp.tile([P, 16 * 65], F32R, tag="va2", name="va2", bufs=2)
                 for _ in range(TI // P)]

        wk_cm, wk = openp(name="wk", bufs=3)
        tok_cm, tokp = openp(name="tok4", bufs=2)
        acckv_cm, acckv = openp(name="psKV", bufs=2, space="PSUM")
        tpi_cm, tpi = openp(name="psT4", bufs=2, space="PSUM")
        for tt in range(TI // P):
            tok = tokp.tile([P, C], F32, tag="tok", name="tok")
            nc.sync.dma_start(out=tok, in_=dr["x_img_feats"].ap()[tt * P:(tt + 1) * P, :])
            for c in range(NCT):
                tps = tpi.tile([P, P], F32, tag="tp", name="tp")
                nc.tensor.transpose(tps, tok[:, c * P:(c + 1) * P], ident)
                nc.vector.tensor_copy(out=imgT[:, c, tt * P:(tt + 1) * P], in_=tps)
        tpi_cm.__exit__(None, None, None)
        tok_cm.__exit__(None, None, None)

        for mc in range(4):
            wt = load_wp("k_p", mc, wk)
            for mh in range(2):
                f = 2 * mc + mh
                kps = acckv.tile([P, 256], F32, tag="acc256", name="acc256")
                dr_group(kps, [(wt[:, kk, :, 128 * mh:128 * mh + 128],
                                imgT[:, 2 * kk:2 * kk + 2, :]) for kk in range(4)])
                nc.scalar.activation(out=k2_t[f], in_=kps, func=AF.Identity,
                                     bias=bk_c[:, f:f + 1], scale=WSI)

        brow_v2 = rowp.tile([1, C], F32, tag="row", name="braw2")
        nc.sync.dma_start(out=brow_v2, in_=dr["bv"].ap().rearrange("(a c) -> a c", a=1))
        bvb2 = bcast_row(brow_v2, wk, acckv, "bvb2")
        for cc in range(4):
            wt = load_wp("v2_p", cc, wk)
            for st in range(TI // P):
                vps = acckv.tile([P, 256], F32, tag="acc256", name="acc256")
                dr_group(vps, [(imgT[:, 2 * kk:2 * kk + 2, st * P:(st + 1) * P],
                                wt[:, kk, :, :]) for kk in range(4)])
                dst = v2aug[st].rearrange("p (h x) -> p h x", x=65)[:, 4 * cc:4 * (cc + 1), 0:64]
                nc.vector.scalar_tensor_tensor(
                    out=dst, in0=vps.rearrange("p (h x) -> p h x", x=64),
                    scalar=WSI,
                    in1=bvb2[:, 256 * cc:256 * (cc + 1)].rearrange("p (h x) -> p h x", x=64),
                    op0=OP.mult, op1=OP.add)
        for st in range(TI // P):
            nc.vector.tensor_copy(
                out=v2aug[st].rearrange("p (h x) -> p h x", x=65)[:, :, 64:65],
                in_=ones_col.rearrange("p (h x) -> p h x", x=1))
        wk_cm.__exit__(None, None, None)

        # ---- self-attention v projection (3-pass) + q,k (1-pass)
        vap_cm, vap = openp(name="vaug", bufs=NTT, side="right")
        vaug = [vap.tile([P, 16 * 65], F32R, tag="va", name="va") for _ in range(NTT)]

        wv_cm, wv = openp(name="wv", bufs=4)
        accv_cm, accv = openp(name="psACv", bufs=2, space="PSUM")
        brow_v = rowp.tile([1, C], F32, tag="row", name="braw")
        nc.sync.dma_start(out=brow_v,
                          in_=dr["b_attn"].ap()[2 * C:3 * C].rearrange("(a c) -> a c", a=1))
        bvb1 = bcast_row(brow_v, wv, accv, "bvb")
        for cc in range(4):   # v output chunks of 256 cols (4 heads each)
            wt = load_wp("attn_p", 8 + cc, wv)
            wtR = load_wp("vR_p", cc, wv)
            for tt in range(NTT):
                vps = accv.tile([P, 256], F32, tag="acc", name="acc")
                tsl = slice(tt * P, (tt + 1) * P)
                ps = ([(x8[:, 2 * kk:2 * kk + 2, tsl], wt[:, kk, :, :]) for kk in range(4)]
                      + [(xd8[:, 2 * kk:2 * kk + 2, tsl], wt[:, kk, :, :]) for kk in range(4)]
                      + [(x8[:, 2 * kk:2 * kk + 2, tsl], wtR[:, kk, :, :]) for kk in range(4)])
                dr_group(vps, ps)
                dst = vaug[tt].rearrange("p (h x) -> p h x", x=65)[:, 4 * cc:4 * (cc + 1), 0:64]
                nc.vector.scalar_tensor_tensor(
                    out=dst, in0=vps.rearrange("p (h x) -> p h x", x=64),
                    scalar=A_col[:, tt:tt + 1],
                    in1=bvb1[:, 256 * cc:256 * (cc + 1)].rearrange("p (h x) -> p h x", x=64),
                    op0=OP.mult, op1=OP.add)
        for tt in range(NTT):
            nc.vector.tensor_copy(
                out=vaug[tt].rearrange("p (h x) -> p h x", x=65)[:, :, 64:65],
                in_=ones_col.rearrange("p (h x) -> p h x", x=1))
        accv_cm.__exit__(None, None, None)
        wv_cm.__exit__(None, None, None)
        acckv_cm.__exit__(None, None, None)
        xd8_cm.__exit__(None, None, None)

        qk_cm, qkp = openp(name="qk", bufs=16, side="right")
        w1_cm, w1 = openp(name="w1", bufs=3)
        acc_cm, accp = openp(name="psAC1", bufs=4, space="PSUM")
        qk_t = []
        for mc in range(8):
            wt = load_wp("attn_p", mc, w1)
            for mh in range(2):
                f = 2 * mc + mh
                qt = qkp.tile([P, T], BF16, tag="qk", name="qk")
                for tch in range(2):
                    sl = slice(512 * tch, 512 * (tch + 1))
                    aps = accp.tile([P, 512], F32, tag="acc", name="acc")
                    dr_group(aps, ws_passes(wt, None, x8, None,
                                            slice(128 * mh, 128 * mh + 128), sl))
                    nc.vector.tensor_tensor(out=qt[:, sl], in0=aps, in1=A_b[:, sl],
                                            op=OP.mult)
                    if qk_bias:
                        nc.vector.tensor_scalar_add(out=qt[:, sl], in0=qt[:, sl],
                                                    scalar1=bqk[:, f:f + 1])
                qk_t.append(qt)
        acc_cm.__exit__(None, None, None)
        w1_cm.__exit__(None, None, None)
        x8_cm.__exit__(None, None, None)

        # ================= P2: self attention =================
        o_cm, opool = openp(name="o1", bufs=1)
        o_all = opool.tile([P, NCT, T], BF16, tag="ot", name="ot")
        pp_cm, pp = openp(name="pp1", bufs=5)
        psS_cm, psS = openp(name="psS1", bufs=2, space="PSUM")

        def kq_self(h):
            return (qk_t[8 + h // 2], (h % 2) * D), (qk_t[h // 2], (h % 2) * D)

        for tch in range(2):
            for h in range(H):
                attn_chunk(kq_self, vaug, 4 * (tch + 1), h, tch, psS, pp,
                           causal=True, o_all=o_all)

        psS_cm.__exit__(None, None, None)
        pp_cm.__exit__(None, None, None)
        qk_cm.__exit__(None, None, None)
        vap_cm.__exit__(None, None, None)

        # ================= P3: aproj (bf16) + residual in place ======
        w2_cm, w2 = openp(name="w2", bufs=3)
        acc_cm, accp = openp(name="psAC3", bufs=3, space="PSUM")
        for co in range(NCT):
            wt = load_wb("aproj_b", co, w2)
            for tch in range(2):
                sl = slice(512 * tch, 512 * (tch + 1))
                aps = accp.tile([P, 512], F32, tag="acc", name="acc")
                for c in range(NCT):
                    nc.tensor.matmul(aps, wt[:, c, :], o_all[:, c, sl],
                                     start=(c == 0), stop=(c == NCT - 1))
                nc.vector.tensor_tensor(out=resid[co][:, sl], in0=aps,
                                        in1=resid[co][:, sl], op=OP.add)
                if ab_bias:
                    nc.vector.tensor_scalar_add(
                        out=resid[co][:, sl], in0=resid[co][:, sl],
                        scalar1=bap_c[:, co:co + 1])
        acc_cm.__exit__(None, None, None)
        w2_cm.__exit__(None, None, None)
        o_cm.__exit__(None, None, None)

        # ================= P4: cross attention projections =================
        x1_cm, x1p = openp(name="x18", bufs=1)
        x18 = x1p.tile([P, NCT, T], F8, tag="x8", name="x8")

        ln_cm, lnp = openp(name="psLN1", bufs=2, space="PSUM")
        A_b, _ = ln_stats(resid, lnp)
        ln_cm.__exit__(None, None, None)
        quant_x(resid, x18, None, pool_only=True)

        w3_cm, w3 = openp(name="w3", bufs=3)
        acc_cm, accp = openp(name="psAC4", bufs=4, space="PSUM")
        q2_cm, q2p = openp(name="q2", bufs=NCT, side="right")
        q2_t = []
        for mc in range(4):
            wt = load_wp("q_p", mc, w3)
            for mh in range(2):
                f = 2 * mc + mh
                qt = q2p.tile([P, T], BF16, tag="q2", name="q2")
                for tch in range(2):
                    sl = slice(512 * tch, 512 * (tch + 1))
                    aps = accp.tile([P, 512], F32, tag="acc", name="acc")
                    dr_group(aps, ws_passes(wt, None, x18, None,
                                            slice(128 * mh, 128 * mh + 128), sl))
                    nc.vector.tensor_tensor(out=qt[:, sl], in0=aps, in1=A_b[:, sl],
                                            op=OP.mult)
                    if q2_bias:
                        nc.vector.tensor_scalar_add(out=qt[:, sl], in0=qt[:, sl],
                                                    scalar1=bq_c[:, f:f + 1])
                q2_t.append(qt)
        acc_cm.__exit__(None, None, None)
        w3_cm.__exit__(None, None, None)
        x1_cm.__exit__(None, None, None)

        # ================= P5: cross attention =================
        o_cm, opool = openp(name="o2", bufs=1)
        o2_all = opool.tile([P, NCT, T], F8, tag="ot", name="ot")
        pp_cm, pp = openp(name="pp2", bufs=4)
        psS_cm, psS = openp(name="psS2", bufs=2, space="PSUM")

        def kq_cross(h):
            return (k2_t[h // 2], (h % 2) * D), (q2_t[h // 2], (h % 2) * D)

        for tch in range(2):
            for h in range(H):
                attn_chunk(kq_cross, v2aug, TI // P, h, tch, psS, pp,
                           causal=False, o_all=o2_all, rb_split=False,
                           s_bufs=2, o_bufs=4)

        psS_cm.__exit__(None, None, None)
        pp_cm.__exit__(None, None, None)
        q2_cm.__exit__(None, None, None)
        cross_cm.__exit__(None, None, None)

        # ================= P6: cproj + residual (x2, in place) =================
        w4_cm, w4 = openp(name="w4", bufs=3)
        acc_cm, accp = openp(name="psAC5", bufs=3, space="PSUM")
        for mc in range(4):
            wt = load_wp("cproj_p", mc, w4)
            for mh in range(2):
                co = 2 * mc + mh
                for tch in range(2):
                    sl = slice(512 * tch, 512 * (tch + 1))
                    aps = accp.tile([P, 512], F32, tag="acc", name="acc")
                    dr_group(aps, ws_passes(wt, None, o2_all, None,
                                            slice(128 * mh, 128 * mh + 128), sl))
                    nc.vector.scalar_tensor_tensor(
                        out=resid[co][:, sl], in0=aps, scalar=WSI,
                        in1=resid[co][:, sl], op0=OP.mult, op1=OP.add)
                    if cp_bias:
                        nc.vector.tensor_scalar_add(
                            out=resid[co][:, sl], in0=resid[co][:, sl],
                            scalar1=bcp_c[:, co:co + 1])
        acc_cm.__exit__(None, None, None)
        w4_cm.__exit__(None, None, None)
        o_cm.__exit__(None, None, None)

        # ================= P7: MLP =================
        x2_cm, x2p = openp(name="x28", bufs=1)
        x28 = x2p.tile([P, NCT, T], F8, tag="x8", name="x8")
        x2d8 = x2p.tile([P, NCT, T], F8, tag="xd8", name="xd8")

        ln_cm, lnp = openp(name="psLN2", bufs=2, space="PSUM")
        A_b, _ = ln_stats(resid, lnp)
        ln_cm.__exit__(None, None, None)
        quant_x(resid, x28, x2d8)

        up_cm, up = openp(name="u", bufs=16, side="right")
        utiles = [up.tile([P, 2, T], BF16, tag="u", name="u") for _ in range(16)]
        uscr_cm, uscrp = openp(name="uscr", bufs=4)
        w5_cm, w5 = openp(name="w5", bufs=4)
        accU_cm, accU = openp(name="psU", bufs=4, space="PSUM")
        for mc in range(16):
            wt = load_wp("fc_p", mc, w5)
            wtR = load_wp("fcR_p", mc, w5)
            for mh in range(2):
                ff = 2 * mc + mh
                for tch in range(2):
                    sl = slice(512 * tch, 512 * (tch + 1))
                    ups = accU.tile([P, 512], F32, tag="acc", name="acc")
                    dr_group(ups, ws_passes(wt, wtR, x28, x2d8,
                                            slice(128 * mh, 128 * mh + 128), sl))
                    uscr = uscrp.tile([P, 512], F32, tag="us", name="us")
                    nc.vector.tensor_tensor(out=uscr, in0=ups, in1=A_b[:, sl],
                                            op=OP.mult)
                    nc.scalar.activation(out=utiles[ff // 2][:, ff % 2, sl], in_=uscr,
                                         func=AF.Gelu_apprx_tanh,
                                         bias=bfc_c[:, ff:ff + 1], scale=1.0)
        accU_cm.__exit__(None, None, None)
        w5_cm.__exit__(None, None, None)
        uscr_cm.__exit__(None, None, None)
        x2_cm.__exit__(None, None, None)

        tok_cm, tokp = openp(name="tok7", bufs=2)
        tp_cm, tpp = openp(name="psT7", bufs=4, space="PSUM")
        w6_cm, w6 = openp(name="w6", bufs=3)
        psM_cm, psM = openp(name="psM", bufs=3, space="PSUM")
        for co in range(NCT):
            wt = load_wb("mproj_b", co, w6)
            for tch in range(2):
                sl = slice(512 * tch, 512 * (tch + 1))
                mps = psM.tile([P, 512], F32, tag="m", name="m")
                for ff in range(FF // P):
                    nc.tensor.matmul(mps, wt[:, ff, :], utiles[ff // 2][:, ff % 2, sl],
                                     start=(ff == 0), stop=(ff == FF // P - 1))
                nc.vector.tensor_tensor(out=resid[co][:, sl], in0=mps,
                                        in1=resid[co][:, sl], op=OP.add)
                if mp_bias:
                    nc.vector.tensor_scalar_add(
                        out=resid[co][:, sl], in0=resid[co][:, sl],
                        scalar1=bmp_c[:, co:co + 1])
        psM_cm.__exit__(None, None, None)
        w6_cm.__exit__(None, None, None)

        # ================= P8: transpose back & store =================
        for tt in range(NTT):
            otok = tokp.tile([P, C], F32, tag="tok", name="tok")
            for c in range(NCT):
                tps = tpp.tile([P, P], F32R, tag="tpr", name="tpr")
                nc.tensor.transpose(tps, resid[c][:, tt * P:(tt + 1) * P], identR)
                if c % 2:
                    nc.vector.tensor_copy(out=otok[:, c * P:(c + 1) * P], in_=tps)
                else:
                    nc.scalar.copy(out=otok[:, c * P:(c + 1) * P], in_=tps)
            nc.sync.dma_start(out=out_d.ap()[tt * P:(tt + 1) * P, :], in_=otok)
        tp_cm.__exit__(None, None, None)
        tok_cm.__exit__(None, None, None)
        up_cm.__exit__(None, None, None)
        res_cm.__exit__(None, None, None)

        for cm in reversed(kw_cms):
            cm.__exit__(None, None, None)

    nc.compile()
    return nc


def _pack_core(Wq, Mc):
    K, M = Wq.shape
    KK, MC = K // 256, M // Mc
    A = Wq.reshape(KK, 2, P, MC, Mc).transpose(3, 2, 0, 1, 4)
    return np.ascontiguousarray(A.reshape(MC, P, KK * 2 * Mc))


def _pack_w(W, Mc=256):
    import ml_dtypes
    return _pack_core((np.asarray(W, np.float32) * WS).astype(ml_dtypes.float8_e4m3), Mc)


def _pack_wr(W, Mc=256):
    import ml_dtypes
    Ws = np.asarray(W, np.float32) * WS
    W8 = Ws.astype(ml_dtypes.float8_e4m3)
    R = Ws - W8.astype(np.float32)
    return _pack_core(R.astype(ml_dtypes.float8_e4m3), Mc)


def _pack_b(W):
    """[K, M] f32 -> [M//128, 128, (K//128)*128] bf16 stationary slabs."""
    import ml_dtypes
    K, M = W.shape
    A = np.asarray(W, np.float32).reshape(K // P, P, M // P, P).transpose(2, 1, 0, 3)
    return np.ascontiguousarray(A.reshape(M // P, P, K).astype(ml_dtypes.bfloat16))


def _fold(W, g):
    Wg = np.asarray(W, np.float32) * np.asarray(g, np.float32)[:, None]
    return Wg - Wg.mean(0, keepdims=True)


def kernel(**inputs):
    from concourse.bass_utils import run_bass_kernel_spmd

    np_inputs = {k: np.asarray(v, dtype=np.float32) for k, v in inputs.items()}
    g1, b1 = np_inputs["ln1_g"], np_inputs["ln1_b"]
    g2, b2 = np_inputs["ln2_g"], np_inputs["ln2_b"]
    Wa = np_inputs["W_attn"]

    # fold LN gains/means into weights; fold LN biases into effective biases
    Wa_f = _fold(Wa, g1)
    Wq_f = _fold(np_inputs["Wq"], g1)
    Wfc_f = _fold(np_inputs["W_fc"], g2)
    b_attn_eff = np_inputs["b_attn"] + Wa.T @ b1
    bq_eff = np_inputs["bq"] + np_inputs["Wq"].T @ b1
    bfc_eff = np_inputs["b_fc"] + np_inputs["W_fc"].T @ b2

    flags = (bool(np.any(b_attn_eff[0:2 * C])), bool(np.any(bq_eff)),
             bool(np.any(np_inputs["b_aproj"])), bool(np.any(np_inputs["bcproj"])),
             bool(np.any(np_inputs["b_mproj"])))
    key = ("nc", flags)
    if key not in _CACHED:
        _CACHED[key] = _build(flags)
    nc = _CACHED[key]

    packed = {
        "attn_p": _pack_w(Wa_f),
        "vR_p": _pack_wr(Wa_f[:, 2 * C:3 * C]),
        "q_p": _pack_w(Wq_f),
        "k_p": _pack_w(np_inputs["Wk"]),
        "v2_p": _pack_w(np_inputs["Wv"]),
        "cproj_p": _pack_w(np_inputs["Wcproj"]),
        "fc_p": _pack_w(Wfc_f),
        "fcR_p": _pack_wr(Wfc_f),
        "aproj_b": _pack_b(np_inputs["W_aproj"]),
        "mproj_b": _pack_b(np_inputs["W_mproj"]),
    }
    small = {
        "b_attn": b_attn_eff, "bq": bq_eff, "b_fc": bfc_eff,
        "b_aproj": np_inputs["b_aproj"], "bk": np_inputs["bk"],
        "bv": np_inputs["bv"], "bcproj": np_inputs["bcproj"],
        "b_mproj": np_inputs["b_mproj"],
    }
    in_maps = []
    for b in range(B):
        m = dict(small)
        m.update(packed)
        m["x"] = np.ascontiguousarray(np_inputs["x"][b])
        m["x_img_feats"] = np.ascontiguousarray(np_inputs["x_img_feats"][b])
        in_maps.append(m)
    res = run_bass_kernel_spmd(nc, in_maps, core_ids=list(range(B)))
    out = np.stack([res.results[b]["out"] for b in range(B)], axis=0)
    return out.astype(np.float32)
